# revision 1
# baseline (speedup 1.0000x reference)
"""DogeDynamicMaskAttention Trainium2 kernel.

Sharding: 8 cores = 2 batches x 4 head-groups. Core c: batch b=c//4,
head-group g=c%4 -> heads [4g..4g+4), kv heads {2g, 2g+1}.

Device program (SPMD; identical program on all cores, different data):
  - q/k/v projections from xT as fp32r matmuls, outputs in transposed
    [out_dim, S] layout; SCALING folded into Wq on host.
  - dt = v_flat @ Wdt.T (all kv heads), dyn = exp(A * softplus(dt)).
  - exact per-head kthvalue threshold via 31-step bisection on float bits
    (dyn > 0 so float bits are monotonic; one fused DVE op per step).
  - RoPE via permutation-matmul rotate-half + DVE combine.
  - full SxS attention per head: scores psum = qk (fp32r) + rank-1 dyn-mask
    row + rank-1 causal-const row, DVE add for the non-constant (diagonal)
    mask blocks; exp with no max-subtraction (masked entries <= -1.7e38 so
    exp == 0); P * (1/l); PE transpose; attn@v; per-head output projection
    partials summed on host.
  - fully-masked (degenerate) rows give l == 0; host detects via the l
    output (and any non-finite rows) and recomputes those rows faithfully
    in numpy; expected count is ~1 row per (batch, head).
"""
import sys
import numpy as np

sys.path.insert(0, "/root/.axon_site/_ro/trn_rl_repo")

import concourse.bass as bass  # noqa: E402,F401
from concourse import bacc  # noqa: E402
import concourse.tile as tile  # noqa: E402
import concourse.mybir as mybir  # noqa: E402
from concourse.bass_utils import run_bass_kernel_spmd  # noqa: E402
from concourse.alu_op_type import AluOpType  # noqa: E402

F32 = mybir.dt.float32
F32R = mybir.dt.float32r
BF16 = mybir.dt.bfloat16
I32 = mybir.dt.int32
AF = mybir.ActivationFunctionType
AX = mybir.AxisListType.X

B, S, HID = 2, 2048, 2048
H, KV, D = 16, 8, 128
HPC, KVPC = 4, 2
GROUPS = H // KV
NUM_DYN = S // 2
SCALING = D ** -0.5
MIN = float(np.finfo(np.float32).min)
BIG = 1.7e38
P = 128
NT = S // P          # 16
NQ = 4
QW = S // NQ         # 512
NCORES = 8

_cache = {}


def _build_program(blkstate):
    key = ("nc", blkstate)
    if key in _cache:
        return _cache[key]
    nc = bacc.Bacc("TRN2", target_bir_lowering=False, debug=False,
                   num_devices=NCORES)
    dram = {}
    for name, shape in [
            ("xT", [HID, S]), ("wqT", [HID, HPC * D]), ("wkT", [HID, KVPC * D]),
            ("wvT", [HID, KVPC * D]), ("wdtvT", [HID, HPC]),
            ("woT", [HPC * D, HID]), ("acol", [HPC, 1]),
            ("cosT", [D, S]), ("sinT", [D, S]),
            ("varblk", [P, NT * P]),
            ("eye", [P, P]), ("perm", [P, P]), ("ones1", [1, P])]:
        dram[name] = nc.dram_tensor(name, shape, F32, kind="ExternalInput").ap()
    outT_d = nc.dram_tensor("outT", [HID, S], F32, kind="ExternalOutput").ap()
    dram["dyn_dr"] = nc.dram_tensor("dyn_dr", [HPC, S], F32R).ap()
    dram["vnat_dr"] = nc.dram_tensor("vnat_dr", [KVPC * NT * P, P], F32R).ap()
    l_d = nc.dram_tensor("l_out", [HPC, S], F32, kind="ExternalOutput").ap()

    with tile.TileContext(nc) as tc:
        _emit(nc, tc, dram, outT_d, l_d, blkstate)
    nc.compile()
    _cache[key] = nc
    return nc


def _emit(nc, tc, dram, outT_d, l_d, blkstate):
    from contextlib import ExitStack
    ctx = ExitStack()
    consts = ctx.enter_context(tc.tile_pool(name="consts", bufs=1))

    def cst(name, shape, src=None, as_f32r=False):
        t = consts.tile(shape, F32, name=f"c_{name}")
        nc.sync.dma_start(t[:], src if src is not None else dram[name])
        if as_f32r:
            r = consts.tile(shape, F32R, name=f"cr_{name}")
            nc.scalar.copy(r[:], t[:])
            return t, r
        return t

    eye_f, eye_r = cst("eye", [P, P], as_f32r=True)
    perm_t = cst("perm", [P, P])
    _, ones1_r = cst("ones1", [1, P], as_f32r=True)
    acol_t = cst("acol", [HPC, 1])
    # wdtvT packed [128, 16*4]: col cc*4+j = wdtvT[cc*128+p, j]
    wdtv_f = consts.tile([P, NT * HPC], F32, name="c_wdtvT")
    nc.sync.dma_start(wdtv_f[:].rearrange("p (c j) -> p c j", c=NT),
                      dram["wdtvT"].rearrange("(c p) j -> p c j", p=P))
    kthc = consts.tile([HPC, 1], F32, name="kthc")
    nc.vector.memset(kthc[:], float(NUM_DYN) - 0.5)

    act = ctx.enter_context(tc.tile_pool(name="act", bufs=1))
    qkro = [act.tile([P, S], F32R, name=f"qro{h}") for h in range(HPC)]
    kro = [act.tile([P, S], F32R, name=f"kro{i}") for i in range(KVPC)]

    with ExitStack() as ctx1:
        vop = ctx1.enter_context(tc.tile_pool(name="vop", bufs=1))
        vT_own = [vop.tile([P, S], F32R, name=f"vTown{i}") for i in range(KVPC)]
        dt_sb = vop.tile([HPC, S], F32, name="dt_sb")
        csp = ctx1.enter_context(tc.tile_pool(name="csp", bufs=1))
        cos_t = csp.tile([D, S], F32, name="cos_t")
        nc.sync.dma_start(cos_t[:], dram["cosT"])
        sin_t = csp.tile([D, S], F32, name="sin_t")
        nc.sync.dma_start(sin_t[:], dram["sinT"])

        # ---------------- dt first (enables early dyn/bisection) --------
        dyq = ctx1.enter_context(tc.tile_pool(name="dyq", bufs=1))
        with tc.tile_pool(name="dts", bufs=4) as dts, \
             tc.tile_pool(name="dps", bufs=2, space="PSUM") as dps:
            for sg in range(4):
                dtp = dps.tile([HPC, QW], F32, name="dtp", tag="dtp")
                for cc in range(NT):
                    x32 = dts.tile([P, QW], F32, name="x32", tag="x32")
                    nc.sync.dma_start(
                        x32[:], dram["xT"][cc * P:(cc + 1) * P,
                                           sg * QW:(sg + 1) * QW])
                    nc.tensor.matmul(dtp[:], wdtv_f[:, cc * HPC:(cc + 1) * HPC],
                                     x32[:], start=(cc == 0), stop=(cc == NT - 1))
                nc.scalar.copy(dt_sb[:, sg * QW:(sg + 1) * QW], dtp[:])

        # ---------------- dyn + kth bisection (overlaps projections) ----
        kth_f = dyq.tile([HPC, 1], I32, name="kth_f")
        dynrow = dyq.tile([HPC, S], F32R, name="dynrow")
        dyn_t = dyq.tile([HPC, S], F32, name="dyn_t")
        work = dyq.tile([HPC, S], F32, name="work")
        scr = dyq.tile([HPC, S], BF16, name="scr")
        scrf = dyq.tile([HPC, S], F32, name="scrf")
        nc.scalar.activation(work[:], dt_sb[:], AF.Exp)
        nc.scalar.activation(work[:], work[:], AF.Ln, bias=1.0)
        nc.scalar.activation(dyn_t[:], work[:], AF.Exp, scale=acol_t[:])
        lo = dyq.tile([HPC, 1], I32, name="lo")
        hi = dyq.tile([HPC, 1], I32, name="hi")
        mid = dyq.tile([HPC, 1], I32, name="mid")
        dlt = dyq.tile([HPC, 1], I32, name="dlt")
        cges = dyq.tile([HPC, 1], I32, name="cges")
        cltv = dyq.tile([HPC, 1], I32, name="cltv")
        cnt = dyq.tile([HPC, 1], F32, name="cnt")
        nc.vector.memset(lo[:], 0)
        nc.vector.memset(hi[:], 0x7F800000)
        for _ in range(31):
            nc.vector.tensor_tensor(dlt[:], hi[:], lo[:], op=AluOpType.subtract)
            nc.vector.tensor_scalar(dlt[:], dlt[:], 1, None,
                                    op0=AluOpType.arith_shift_right)
            nc.vector.tensor_tensor(mid[:], dlt[:], lo[:], op=AluOpType.add)
            nc.vector.tensor_scalar(scr[:], dyn_t[:],
                                    mid[:, 0:1].bitcast(F32), 0.0,
                                    op0=AluOpType.is_lt, op1=AluOpType.add,
                                    accum_out=cnt[:])
            nc.vector.tensor_scalar(cges[:], kthc[:], cnt[:, 0:1], None,
                                    op0=AluOpType.is_lt)
            nc.vector.tensor_scalar(cltv[:], kthc[:], cnt[:, 0:1], None,
                                    op0=AluOpType.is_ge)
            nc.vector.copy_predicated(hi[:], cges[:], mid[:])
            nc.vector.copy_predicated(lo[:], cltv[:], mid[:])
        nc.vector.tensor_copy(kth_f[:], lo[:])
        pen = scrf
        nc.vector.tensor_scalar(pen[:], dyn_t[:],
                                kth_f[:, 0:1].bitcast(F32), -BIG,
                                op0=AluOpType.is_lt, op1=AluOpType.mult)
        nc.vector.tensor_tensor(dynrow[:], dyn_t[:], pen[:], op=AluOpType.add)
        nc.sync.dma_start(dram["dyn_dr"], dynrow[:])

        # ---------------- projections ----------------
        with tc.tile_pool(name="xp", bufs=1) as xp, \
             tc.tile_pool(name="wp", bufs=2) as wp, \
             tc.tile_pool(name="pjp", bufs=5) as pjp, \
             tc.tile_pool(name="pps", bufs=8, space="PSUM") as pps:
            wname = {"v": "wvT", "q": "wqT", "k": "wkT"}
            OT = ([("v", i) for i in range(KVPC)]
                  + [("q", i) for i in range(HPC)]
                  + [("k", i) for i in range(KVPC)])
            for sg in range(4):
                xfull = xp.tile([P, NT * QW], F32R, name="xfull", tag="xf")
                nc.gpsimd.dma_start(
                    xfull[:].rearrange("p (c f) -> p c f", c=NT),
                    dram["xT"][:, sg * QW:(sg + 1) * QW]
                    .rearrange("(c p) f -> p c f", p=P))
                for kind, oi in OT:
                    wfull = wp.tile([P, NT * P], F32R, name="wfull", tag="wf")
                    nc.gpsimd.dma_start(
                        wfull[:].rearrange("p (c f) -> p c f", c=NT),
                        dram[wname[kind]][:, oi * P:(oi + 1) * P]
                        .rearrange("(c p) f -> p c f", p=P))
                    ps = pps.tile([P, QW], F32, name="ps", tag="ps")
                    for cc in range(NT):
                        nc.tensor.matmul(ps[:], wfull[:, cc * P:(cc + 1) * P],
                                         xfull[:, cc * QW:(cc + 1) * QW],
                                         start=(cc == 0), stop=(cc == NT - 1))
                    if kind == "v":
                        dst = vT_own[oi][:, sg * QW:(sg + 1) * QW]
                        nc.scalar.copy(dst, ps[:])
                    else:
                        f32t = pjp.tile([P, QW], F32, name="pj32", tag="pj")
                        nc.scalar.copy(f32t[:], ps[:])
                        dstro = (qkro[oi] if kind == "q" else kro[oi])
                        rh = pps.tile([P, QW], F32, name="rh", tag="ps")
                        nc.tensor.matmul(rh[:], perm_t[:], f32t[:],
                                         start=True, stop=True)
                        t1 = pjp.tile([P, QW], F32, name="t1", tag="pj")
                        nc.vector.tensor_tensor(
                            t1[:], rh[:], sin_t[:, sg * QW:(sg + 1) * QW],
                            op=AluOpType.mult)
                        t2 = pjp.tile([P, QW], F32, name="t2", tag="pj")
                        nc.vector.tensor_tensor(
                            t2[:], f32t[:], cos_t[:, sg * QW:(sg + 1) * QW],
                            op=AluOpType.mult)
                        nc.vector.tensor_tensor(
                            dstro[:, sg * QW:(sg + 1) * QW], t1[:], t2[:],
                            op=AluOpType.add)

        # ---------------- natural-layout v tiles (bounced via DRAM) ------
        with tc.tile_pool(name="vnb", bufs=4) as vnb, \
             tc.tile_pool(name="vps", bufs=4, space="PSUM") as vps:
            for i in range(KVPC):
                for cc in range(NT):
                    pt = vps.tile([P, P], F32, name="vt", tag="vt")
                    nc.tensor.transpose(pt[:].bitcast(F32R),
                                        vT_own[i][:, cc * P:(cc + 1) * P],
                                        eye_r[:])
                    vn = vnb.tile([P, P], F32R, name="vn", tag="vn")
                    nc.scalar.copy(vn[:], pt[:])
                    nc.sync.dma_start(
                        dram["vnat_dr"][(i * NT + cc) * P:(i * NT + cc + 1) * P, :],
                        vn[:])

    # ---------------- attention ----------------
    # blkstate[t][j] in {"Z", "M", "V:<idx>"}: zero / masked-const / varying
    # computed extent per tile: up to last non-M block
    ext = []
    for t in range(NT):
        nz = [j for j in range(NT) if blkstate[t][j] != "M"]
        ext.append((max(nz) + 1) * P if nz else 0)
    ares = ctx.enter_context(tc.tile_pool(name="ares", bufs=1))
    attnT = [ares.tile([P, S], F32R, name=f"attnT{h}") for h in range(HPC)]
    dynrow0 = [ares.tile([1, S], F32R, name=f"dynrow0_{h}") for h in range(HPC)]
    varblk_t = ares.tile([P, NT * P], F32, name="varblk_t")
    nc.sync.dma_start(varblk_t[:], dram["varblk"])
    for h in range(HPC):
        nc.sync.dma_start(dynrow0[h][:], dram["dyn_dr"][h:h + 1, :])
    with tc.tile_pool(name="ppl", bufs=6) as ppl, \
         tc.tile_pool(name="lpl", bufs=16) as lpl, \
         tc.tile_pool(name="ptl", bufs=6) as ptl, \
         tc.tile_pool(name="vnl", bufs=8) as vnl, \
         tc.tile_pool(name="aps", bufs=6, space="PSUM") as aps, \
         tc.tile_pool(name="ovl", bufs=2, space="PSUM") as ovl:
        for h in range(HPC):
            kv = h // GROUPS
            for grp in range(4):
                glim = max(ext[grp * 4 + tq] for tq in range(4))
                glim = ((glim + QW - 1) // QW) * QW  # pad group extent to 512
                ptiles = []
                for tq in range(4):
                    t = grp * 4 + tq
                    ptile = ppl.tile([P, S], F32R, name="ptile", tag="pt")
                    lparts = lpl.tile([P, NQ], F32, name="lparts", tag="lp")
                    nc.vector.memset(lparts[:], 0.0)
                    for qq in range(NQ):
                        q0 = qq * QW
                        e = min(max(ext[t] - q0, 0), QW)
                        if q0 >= glim:
                            break  # rest of group never read
                        if e == 0:
                            nc.vector.memset(ptile[:, q0:min(q0 + QW, glim)].bitcast(F32), 0.0)
                            nc.vector.memset(lparts[:, qq:qq + 1], 0.0)
                            continue
                        sc = aps.tile([P, QW], F32, name="sc", tag="aps")
                        nc.tensor.matmul(
                            sc[:, :e], qkro[h][:, t * P:(t + 1) * P],
                            kro[kv][:, q0:q0 + e],
                            start=True, stop=True, skip_group_check=True)
                        nc.tensor.matmul(
                            sc[:, :e], ones1_r[:], dynrow0[h][:, q0:q0 + e],
                            start=False, stop=True, skip_group_check=True)
                        for j in range(q0 // P, (q0 + e) // P):
                            st = blkstate[t][j]
                            if st.startswith("V"):
                                vi = int(st[2:])
                                off = j * P - q0
                                nc.vector.tensor_tensor(
                                    sc[:, off:off + P], sc[:, off:off + P],
                                    varblk_t[:, vi * P:(vi + 1) * P],
                                    op=AluOpType.add)
                        nc.scalar.activation(
                            ptile[:, q0:q0 + e], sc[:, :e], AF.Exp,
                            accum_out=lparts[:, qq:qq + 1])
                        if e < QW and q0 + e < glim:
                            nc.vector.memset(
                                ptile[:, q0 + e:min(q0 + QW, glim)]
                                .bitcast(F32), 0.0)
                    lsum = lpl.tile([P, 1], F32, name="lsum", tag="ls")
                    nc.vector.reduce_sum(lsum[:], lparts[:], axis=AX)
                    nc.sync.dma_start(
                        l_d[h:h + 1, t * P:(t + 1) * P].rearrange("a b -> b a"),
                        lsum[:])
                    linv = lpl.tile([P, 1], F32, name="linv", tag="ls")
                    nc.vector.reciprocal(linv[:], lsum[:])
                    nc.vector.tensor_scalar(ptile[:, :glim], ptile[:, :glim],
                                            linv[:, 0:1],
                                            None, op0=AluOpType.mult)
                    ptiles.append(ptile)
                ovp = ovl.tile([P, QW], F32, name="ovp", tag="ovp")
                nch = glim // P
                for cc in range(nch):
                    ptt = aps.tile([P, QW], F32, name="ptt", tag="aps")
                    for tq in range(4):
                        nc.tensor.transpose(
                            ptt[:, tq * P:(tq + 1) * P].bitcast(F32R),
                            ptiles[tq][:, cc * P:(cc + 1) * P], eye_r[:])
                    pts = ptl.tile([P, QW], F32R, name="pts", tag="pts")
                    nc.vector.tensor_copy(pts[:], ptt[:])
                    vn = vnl.tile([P, P], F32R, name="vnt", tag="vnt")
                    nc.sync.dma_start(
                        vn[:], dram["vnat_dr"]
                        [(kv * NT + cc) * P:(kv * NT + cc + 1) * P, :])
                    nc.tensor.matmul(ovp[:], vn[:], pts[:],
                                     start=(cc == 0), stop=(cc == nch - 1),
                                     skip_group_check=True)
                nc.scalar.copy(attnT[h][:, grp * QW:(grp + 1) * QW], ovp[:])

    # ---------------- output projection ----------------
    with tc.tile_pool(name="wol", bufs=2) as wol, \
         tc.tile_pool(name="oub", bufs=4) as oub, \
         tc.tile_pool(name="ops", bufs=4, space="PSUM") as ops:
        for ht in range(NT):
            wo = wol.tile([P, HPC * P], F32R, name="wo", tag="wo")
            nc.gpsimd.dma_start(
                wo[:].rearrange("p (h f) -> p h f", h=HPC),
                dram["woT"][:, ht * P:(ht + 1) * P]
                .rearrange("(h p) f -> p h f", p=P))
            for sg in range(4):
                op = ops.tile([P, QW], F32, name="op", tag="op")
                for h in range(HPC):
                    nc.tensor.matmul(op[:], wo[:, h * P:(h + 1) * P],
                                     attnT[h][:, sg * QW:(sg + 1) * QW],
                                     start=(h == 0), stop=(h == HPC - 1))
                ot = oub.tile([P, QW], F32, name="ot", tag="ot")
                nc.scalar.copy(ot[:], op[:])
                nc.sync.dma_start(
                    outT_d[ht * P:(ht + 1) * P, sg * QW:(sg + 1) * QW], ot[:])
    ctx.close()


def _host_prep(hidden_states, cos, sin, attention_mask, Wq, Wk, Wv, A, Wdt, Wo):
    eye = np.eye(P, dtype=np.float32)
    perm = np.zeros((P, P), dtype=np.float32)
    for j in range(64):
        perm[j + 64, j] = -1.0
        perm[j, j + 64] = 1.0
    ones1 = np.ones((1, P), dtype=np.float32)

    in_maps = []
    blkstates = []
    for c in range(NCORES):
        b, g = divmod(c, 4)
        heads = list(range(4 * g, 4 * g + 4))
        wvT = np.ascontiguousarray(Wv[2 * g * D:(2 * g + 2) * D].T)
        wdtvT = np.ascontiguousarray(
            (Wdt[heads].astype(np.float64) @ Wv.astype(np.float64))
            .T.astype(np.float32))
        xT = np.ascontiguousarray(hidden_states[b].T)
        wqT = np.ascontiguousarray(
            (Wq[4 * g * D:(4 * g + 4) * D] * np.float32(SCALING)).T)
        wkT = np.ascontiguousarray(Wk[2 * g * D:(2 * g + 2) * D].T)
        woT = np.ascontiguousarray(Wo[:, 4 * g * D:(4 * g + 4) * D].T)
        acol = A[heads].astype(np.float32).reshape(HPC, 1)
        cosT = np.ascontiguousarray(cos[b].T)
        sinT = np.ascontiguousarray(sin[b].T)
        m = attention_mask[b, 0]
        mb = m.reshape(NT, P, NT, P)
        blkrows = []
        varlist = []
        for t in range(NT):
            row = []
            for j in range(NT):
                blkv = mb[t, :, j, :]
                if np.all(blkv == 0):
                    row.append("Z")
                elif np.all(blkv <= -1e30):
                    row.append("M")
                else:
                    row.append(f"V:{len(varlist)}")
                    varlist.append(np.maximum(blkv, -BIG))
            # interior M blocks (before a later non-M block) become varying
            nz = [j for j in range(NT) if row[j] != "M"]
            lim = (max(nz) + 1) if nz else 0
            for j in range(lim):
                if row[j] == "M":
                    row[j] = f"V:{len(varlist)}"
                    varlist.append(np.full((P, P), -BIG, np.float32))
            blkrows.append(tuple(row))
        if len(varlist) > NT:
            raise NotImplementedError("too many varying mask blocks")
        varblk = np.zeros((P, NT * P), dtype=np.float32)
        for vi, blkv in enumerate(varlist):
            varblk[:, vi * P:(vi + 1) * P] = blkv
        blkstate = tuple(blkrows)
        in_maps.append({
            "xT": xT, "wqT": wqT, "wkT": wkT, "wvT": wvT, "wdtvT": wdtvT,
            "woT": woT, "acol": acol, "cosT": cosT, "sinT": sinT,
            "varblk": varblk, "eye": eye, "perm": perm,
            "ones1": ones1,
        })
        blkstates.append(blkstate)
    if len(set(blkstates)) != 1:
        raise NotImplementedError("mask structure differs across batches")
    return in_maps, blkstates[0]


def _softplus64(x):
    x = x.astype(np.float64)
    return np.log1p(np.exp(-np.abs(x))) + np.maximum(x, 0)


def _repair_rows(out, bad, inputs):
    """Recompute rows flagged bad [B, S] with faithful numpy reference math."""
    if not bad.any():
        return out
    hs = inputs["hidden_states"]; cos = inputs["cos"]; sin = inputs["sin"]
    am = inputs["attention_mask"]; Wq = inputs["Wq"]; Wk = inputs["Wk"]
    Wv = inputs["Wv"]; A = inputs["A"]; Wdt = inputs["Wdt"]; Wo = inputs["Wo"]

    def rope(x, c, s):
        x1, x2 = x[..., :D // 2], x[..., D // 2:]
        return x * c + np.concatenate([-x2, x1], axis=-1) * s

    for b in range(B):
        rows = np.where(bad[b])[0]
        if len(rows) == 0:
            continue
        x = hs[b].astype(np.float32)
        k = (x @ Wk.T).reshape(S, KV, D)
        v = (x @ Wv.T).reshape(S, KV, D)
        k = rope(k, cos[b][:, None, :], sin[b][:, None, :])
        v_flat = v.reshape(S, KV * D)
        dt = v_flat @ Wdt.T
        dyn = np.exp(A[None, :] * _softplus64(dt)).astype(np.float32).T
        kth = np.sort(dyn, axis=-1)[:, NUM_DYN - 1:NUM_DYN]
        dmask = np.where(dyn < kth, MIN, dyn).astype(np.float32)
        for s_i in rows:
            q_row = (x[s_i] @ Wq.T).reshape(H, D)
            q_row = rope(q_row, cos[b][s_i][None, :], sin[b][s_i][None, :])
            attn_row = np.zeros((H, D), dtype=np.float32)
            for h in range(H):
                kvh = h // GROUPS
                sc = ((q_row[h] @ k[:, kvh].T) * np.float32(SCALING)
                      + (dmask[h] + am[b, 0, s_i])).astype(np.float32)
                w = np.exp(sc - sc.max())
                w = (w / w.sum()).astype(np.float32)
                attn_row[h] = w @ v[:, kvh]
            out[b, s_i] = attn_row.reshape(H * D) @ Wo.T
    return out


def kernel(**inputs):
    inputs = {k: np.asarray(v) for k, v in inputs.items()}
    in_maps, blkstate = _host_prep(**inputs)
    nc = _build_program(blkstate)
    res = run_bass_kernel_spmd(nc, in_maps, list(range(NCORES)))
    out = np.zeros((B, S, HID), dtype=np.float32)
    bad = np.zeros((B, S), dtype=bool)
    for c in range(NCORES):
        b = c // 4
        out[b] += res.results[c]["outT"].T
        bad[b] |= (res.results[c]["l_out"] == 0).any(axis=0)
    bad |= ~np.isfinite(out).all(axis=2)
    out = _repair_rows(out, bad, inputs)
    return out



# revision 20
# speedup vs baseline: 1.3535x; 1.3535x over previous
"""DogeDynamicMaskAttention Trainium2 kernel (v2 — transposed attention).

Sharding: 8 cores = 2 batches x 4 head-groups. Core c: batch b=c//4,
head-group g=c%4 -> heads [4g..4g+4), kv heads {2g, 2g+1}.

Device program (SPMD; identical program on all cores, different data):
  - Phase A: dt pre-pass (dt = x @ (Wdt@Wv).T folded on host) streaming x.
  - Phase B: exact per-head kthvalue via 31-step float-bit bisection on DVE
    (drains while phase C runs on PE).
  - Phase C: q/k/v projections with SBUF-resident weights, x streamed once
    per pass as [128,256] tiles; RoPE via f32r permutation-matmul
    rotate-half + DVE combine.
  - Phase D: attention in TRANSPOSED orientation: scT[k,q] = k.q per k-tile,
    causal diag via one constant triangle DVE add, dynamic mask applied as a
    per-partition bias inside the exp activation; attn@v and the softmax
    denominator l (all-ones matmul) accumulate in PSUM over k-tiles.
    Normalization by 1/l via a per-tile transpose/scale/transpose dance.
  - Phase E: output projection out[q,hid] = sum_h attnT_h(t)^T @ WoT_h,
    interleaved per q-chunk with phase D.
  - Degenerate rows (l==0 or non-finite) repaired on host; partials summed
    on host across the 4 head-group cores per batch.
"""
import sys
import numpy as np

sys.path.insert(0, "/root/.axon_site/_ro/trn_rl_repo")

import concourse.bass as bass  # noqa: E402,F401
from concourse import bacc  # noqa: E402
import concourse.tile as tile  # noqa: E402
import concourse.mybir as mybir  # noqa: E402
from concourse.bass_utils import run_bass_kernel_spmd  # noqa: E402
from concourse.alu_op_type import AluOpType  # noqa: E402

F32 = mybir.dt.float32
F32R = mybir.dt.float32r
BF16 = mybir.dt.bfloat16
I32 = mybir.dt.int32
AF = mybir.ActivationFunctionType
AX = mybir.AxisListType.X

B, S, HID = 2, 2048, 2048
H, KV, D = 16, 8, 128
HPC, KVPC = 4, 2
GROUPS = H // KV
NUM_DYN = S // 2
SCALING = D ** -0.5
MIN = float(np.finfo(np.float32).min)
BIG = 1.7e38
P = 128
NT = S // P          # 16 k-tiles
NC = 4               # q chunks of 512
QW = S // NC         # 512
CW = 256             # projection column chunk
NCH = S // CW        # 8 projection chunks
NCORES = 8

_cache = {}


def _build_program(blkstate=None):
    key = "nc"
    if key in _cache:
        return _cache[key]
    nc = bacc.Bacc("TRN2", target_bir_lowering=False, debug=False,
                   num_devices=NCORES)
    dram = {}
    for name, shape, dt_ in [
            ("xT", [HID, S], F32R), ("wqT", [HID, HPC * D], F32R),
            ("wkT", [HID, KVPC * D], F32R), ("wvT", [HID, KVPC * D], F32R),
            ("wdtvT", [HID, HPC], F32R),
            ("woT", [HPC * D, HID], F32R), ("acol", [HPC, 1], F32),
            ("cosT", [D, S], F32), ("sinT", [D, S], F32),
            ("eye", [P, P], F32), ("perm", [P, P], F32), ("tri", [P, P], F32),
            ("ones128", [P, P], F32)]:
        dram[name] = nc.dram_tensor(name, shape, dt_,
                                    kind="ExternalInput").ap()
    out_d = nc.dram_tensor("out_q", [S, HID], F32, kind="ExternalOutput").ap()
    l_d = nc.dram_tensor("l_out", [HPC, S], F32, kind="ExternalOutput").ap()

    with tile.TileContext(nc) as tc:
        _emit(nc, tc, dram, out_d, l_d)
    nc.compile()
    _cache[key] = nc
    return nc


def _emit(nc, tc, dram, out_d, l_d):
    from contextlib import ExitStack
    ctx = ExitStack()
    consts = ctx.enter_context(tc.tile_pool(name="consts", bufs=1))

    def cst(name, shape, as_f32r=False):
        t = consts.tile(shape, F32, name=f"c_{name}")
        nc.sync.dma_start(t[:], dram[name])
        if as_f32r:
            r = consts.tile(shape, F32R, name=f"cr_{name}")
            nc.scalar.copy(r[:], t[:])
            return t, r
        return t

    eye_f, eye_r = cst("eye", [P, P], as_f32r=True)
    _, perm_r = cst("perm", [P, P], as_f32r=True)
    tri_t = cst("tri", [P, P])
    _, ones_r = cst("ones128", [P, P], as_f32r=True)
    acol_t = cst("acol", [HPC, 1])
    # wdtvT packed [128, 16*4]: col cc*4+j = wdtvT[cc*128+p, j]
    wdtv_f = consts.tile([P, NT * HPC], F32R, name="c_wdtvT")
    nc.sync.dma_start(wdtv_f[:].rearrange("p (c j) -> p c j", c=NT),
                      dram["wdtvT"].rearrange("(c p) j -> p c j", p=P))
    kthc = consts.tile([HPC, 1], F32, name="kthc")
    nc.vector.memset(kthc[:], float(NUM_DYN) - 0.5)
    cos_t = consts.tile([D, S], F32, name="cos_t")
    nc.sync.dma_start(cos_t[:], dram["cosT"])
    sin_t = consts.tile([D, S], F32, name="sin_t")
    nc.sync.dma_start(sin_t[:], dram["sinT"])

    # persistent activation tiles
    act = ctx.enter_context(tc.tile_pool(name="act", bufs=1))
    qkro = [act.tile([P, S], F32R, name=f"qro{h}") for h in range(HPC)]
    kro = [act.tile([P, S], F32R, name=f"kro{i}") for i in range(KVPC)]
    dyncol = act.tile([P, NT * HPC], F32, name="dyncol")  # col 4*j+h
    linvc = [act.tile([P, NT], F32, name=f"linvc{h}") for h in range(HPC)]

    # dynm + vT_own live until mid-kernel; kept on ctx (LIFO-friendly)
    dynm = ctx.enter_context(tc.tile_pool(name="dynp", bufs=1)) \
        .tile([HPC, S], F32, name="dynm")
    vto = ctx.enter_context(tc.tile_pool(name="vto", bufs=1))
    vT_own = [vto.tile([P, S], F32R, name=f"vTown{i}") for i in range(KVPC)]

    wctx = ExitStack()  # weights: freed after phase C
    wres = wctx.enter_context(tc.tile_pool(name="wres", bufs=1))
    wqr = wres.tile([P, NT * HPC * P], F32R, name="wqr")
    wkr = wres.tile([P, NT * KVPC * P], F32R, name="wkr")
    wvr = wres.tile([P, NT * KVPC * P], F32R, name="wvr")

    dyqa = ExitStack()  # bisection scratch: freed before phase C
    dyp = dyqa.enter_context(tc.tile_pool(name="dyqa", bufs=1))
    dt_sb = dyp.tile([HPC, S], F32, name="dt_sb")

    # ---------------- Phase A: dt pre-pass (streams x) ----------------
    with tc.tile_pool(name="xa", bufs=4) as xa, \
         tc.tile_pool(name="dps", bufs=2, space="PSUM") as dps:
        for sg in range(4):
            dtp = dps.tile([HPC, QW], F32, name="dtp", tag="dtp")
            for cc in range(NT):
                x32 = xa.tile([P, QW], F32R, name="x32", tag="xa")
                nc.gpsimd.dma_start(
                    x32[:], dram["xT"][cc * P:(cc + 1) * P,
                                       sg * QW:(sg + 1) * QW])
                nc.tensor.matmul(dtp[:], wdtv_f[:, cc * HPC:(cc + 1) * HPC],
                                 x32[:], start=(cc == 0), stop=(cc == NT - 1))
            nc.scalar.copy(dt_sb[:, sg * QW:(sg + 1) * QW], dtp[:])

    # resident weight loads (sync queue; DMA overlaps phase A x stream)
    nc.sync.dma_start(wqr[:].rearrange("p (c f) -> p c f", c=NT),
                      dram["wqT"].rearrange("(c p) f -> p c f", p=P))
    nc.sync.dma_start(wkr[:].rearrange("p (c f) -> p c f", c=NT),
                      dram["wkT"].rearrange("(c p) f -> p c f", p=P))
    nc.sync.dma_start(wvr[:].rearrange("p (c f) -> p c f", c=NT),
                      dram["wvT"].rearrange("(c p) f -> p c f", p=P))

    # ---------------- Phase B: dyn + kth bisection (DVE) -------------
    kth_f = dyp.tile([HPC, 1], I32, name="kth_f")
    dyn_t = dyp.tile([HPC, S], F32, name="dyn_t")
    work = dyp.tile([HPC, S], F32, name="work")
    scr = dyp.tile([HPC, S], BF16, name="scr")
    nc.scalar.activation(work[:], dt_sb[:], AF.Exp)
    nc.scalar.activation(work[:], work[:], AF.Ln, bias=1.0)
    nc.scalar.activation(dyn_t[:], work[:], AF.Exp, scale=acol_t[:])
    lo = dyp.tile([HPC, 1], I32, name="lo")
    hi = dyp.tile([HPC, 1], I32, name="hi")
    mid = dyp.tile([HPC, 1], I32, name="mid")
    dlt = dyp.tile([HPC, 1], I32, name="dlt")
    cges = dyp.tile([HPC, 1], I32, name="cges")
    cltv = dyp.tile([HPC, 1], I32, name="cltv")
    cnt = dyp.tile([HPC, 1], F32, name="cnt")
    nc.vector.memset(lo[:], 0)
    nc.vector.memset(hi[:], 0x7F800000)
    for _ in range(31):
        nc.vector.tensor_tensor(dlt[:], hi[:], lo[:], op=AluOpType.subtract)
        nc.vector.tensor_scalar(dlt[:], dlt[:], 1, None,
                                op0=AluOpType.arith_shift_right)
        nc.vector.tensor_tensor(mid[:], dlt[:], lo[:], op=AluOpType.add)
        nc.vector.tensor_scalar(scr[:], dyn_t[:],
                                mid[:, 0:1].bitcast(F32), 0.0,
                                op0=AluOpType.is_lt, op1=AluOpType.add,
                                accum_out=cnt[:])
        nc.vector.tensor_scalar(cges[:], kthc[:], cnt[:, 0:1], None,
                                op0=AluOpType.is_lt)
        nc.vector.tensor_scalar(cltv[:], kthc[:], cnt[:, 0:1], None,
                                op0=AluOpType.is_ge)
        nc.vector.copy_predicated(hi[:], cges[:], mid[:])
        nc.vector.copy_predicated(lo[:], cltv[:], mid[:])
    nc.vector.tensor_copy(kth_f[:], lo[:])
    # dynm = dyn + (dyn < kth) * (-BIG)   (reuse `work` as scratch)
    nc.vector.tensor_scalar(work[:], dyn_t[:],
                            kth_f[:, 0:1].bitcast(F32), -BIG,
                            op0=AluOpType.is_lt, op1=AluOpType.mult)
    nc.vector.tensor_tensor(dynm[:], dyn_t[:], work[:], op=AluOpType.add)
    dyqa.close()  # free bisection scratch before phase C

    # ---------------- Phase C: projections + RoPE --------------------
    WQW = HPC * P    # 512 cols per cc block in wqr
    WKW = KVPC * P   # 256 cols per cc block in wkr/wvr
    waves = [[("q", 0), ("q", 1), ("q", 2), ("q", 3)],
             [("k", 0), ("k", 1), ("v", 0), ("v", 1)]]
    with tc.tile_pool(name="xc", bufs=20) as xc, \
         tc.tile_pool(name="pj", bufs=6) as pj, \
         tc.tile_pool(name="pps", bufs=5, space="PSUM") as pps, \
         tc.tile_pool(name="rps", bufs=2, space="PSUM") as rps:
        for ch in range(NCH):
            c0 = ch * CW
            xcc = []
            for cc in range(NT):
                xt = xc.tile([P, CW], F32R, name="xcc", tag="xc")
                nc.gpsimd.dma_start(
                    xt[:], dram["xT"][cc * P:(cc + 1) * P, c0:c0 + CW])
                xcc.append(xt)
            for wave in waves:
                for kind, oi in wave:
                    if kind == "q":
                        wsrc, ww = wqr, WQW
                    elif kind == "k":
                        wsrc, ww = wkr, WKW
                    else:
                        wsrc, ww = wvr, WKW
                    pp = pps.tile([P, CW], F32, name="pp", tag="pp")
                    for cc in range(NT):
                        nc.tensor.matmul(
                            pp[:],
                            wsrc[:, cc * ww + oi * P:cc * ww + (oi + 1) * P],
                            xcc[cc][:], start=(cc == 0), stop=(cc == NT - 1),
                            skip_group_check=True)
                    if kind == "v":
                        nc.scalar.copy(vT_own[oi][:, c0:c0 + CW], pp[:])
                        continue
                    f32t = pj.tile([P, CW], F32R, name="pjr", tag="pj")
                    nc.scalar.copy(f32t[:], pp[:])
                    rh = rps.tile([P, CW], F32, name="rh", tag="rh")
                    nc.tensor.matmul(rh[:], perm_r[:], f32t[:],
                                     start=True, stop=True,
                                     skip_group_check=True)
                    t1 = pj.tile([P, CW], F32, name="t1", tag="pj")
                    nc.vector.tensor_tensor(
                        t1[:], rh[:], sin_t[:, c0:c0 + CW], op=AluOpType.mult)
                    t2 = pj.tile([P, CW], F32, name="t2", tag="pj")
                    nc.vector.tensor_tensor(
                        t2[:], f32t[:], cos_t[:, c0:c0 + CW],
                        op=AluOpType.mult)
                    dstro = (qkro[oi] if kind == "q" else kro[oi])
                    nc.vector.tensor_tensor(
                        dstro[:, c0:c0 + CW], t1[:], t2[:], op=AluOpType.add)
    wctx.close()  # free weight SBUF

    # woT tiles for output projection (DMA overlaps phase D start)
    wop = ctx.enter_context(tc.tile_pool(name="wop", bufs=1))
    wot = [wop.tile([P, HID], F32R, name=f"wot{h}") for h in range(HPC)]
    for h in range(HPC):
        nc.sync.dma_start(wot[h][:], dram["woT"][h * P:(h + 1) * P, :])
    vnp = ctx.enter_context(tc.tile_pool(name="vnp", bufs=1))
    vnat = [vnp.tile([P, NT * P], F32R, name=f"vnat{i}") for i in range(KVPC)]

    # natural-layout v tiles + dyncol transposes (PE, after phase C)
    with tc.tile_pool(name="vps", bufs=2, space="PSUM") as vps, \
         tc.tile_pool(name="dcp", bufs=1, space="PSUM") as dcp:
        for i in range(KVPC):
            for j in range(NT):
                pt = vps.tile([P, P], F32, name="vt", tag="vt")
                nc.tensor.transpose(pt[:].bitcast(F32R),
                                    vT_own[i][:, j * P:(j + 1) * P],
                                    eye_r[:])
                nc.scalar.copy(vnat[i][:, j * P:(j + 1) * P], pt[:])
        dct = dcp.tile([P, NT * HPC], F32, name="dct")
        for j in range(NT):
            nc.tensor.transpose(dct[:, j * HPC:(j + 1) * HPC],
                                dynm[:, j * P:(j + 1) * P],
                                eye_f[0:HPC, 0:HPC])
        nc.scalar.copy(dyncol[:], dct[:])

    # ---------------- Phase D/E: attention + output projection -------
    attnT = {}
    with tc.tile_pool(name="ptl", bufs=4) as ptl, \
         tc.tile_pool(name="atl", bufs=9) as atl, \
         tc.tile_pool(name="lrc", bufs=2) as lrc, \
         tc.tile_pool(name="stg", bufs=2) as stg, \
         tc.tile_pool(name="scp", bufs=2, space="PSUM") as scp, \
         tc.tile_pool(name="avp", bufs=2, space="PSUM") as avp, \
         tc.tile_pool(name="llp", bufs=1, space="PSUM") as llp, \
         tc.tile_pool(name="dnp", bufs=1, space="PSUM") as dnp, \
         tc.tile_pool(name="opp", bufs=2, space="PSUM") as opp:
        for c in range(NC):
            jmax = 4 * c + 4
            for h in range(HPC):
                kv = h // GROUPS
                avo = avp.tile([P, QW], F32, name="avo", tag="avo")
                lre = llp.tile([P, QW], F32, name="lre", tag="lre")
                pts = []

                def av_l(jj):
                    ptp, pl0, pe_ = pts[jj]
                    nc.tensor.matmul(
                        avo[:, pl0:pl0 + pe_],
                        vnat[kv][:, jj * P:(jj + 1) * P],
                        ptp[:, pl0:pl0 + pe_],
                        start=(jj == 0), stop=(jj == jmax - 1),
                        skip_group_check=True)
                    nc.tensor.matmul(
                        lre[:, pl0:pl0 + pe_], ones_r[:],
                        ptp[:, pl0:pl0 + pe_],
                        start=(jj == 0), stop=(jj == jmax - 1),
                        skip_group_check=True)

                # software-pipelined: av/l for j-1 emitted after kq/exp of j
                for j in range(jmax):
                    q0 = max(c * QW, j * P)
                    e = (c + 1) * QW - q0
                    loc0 = q0 - c * QW
                    sc = scp.tile([P, QW], F32, name="sc", tag="sc")
                    nc.tensor.matmul(
                        sc[:, :e], kro[kv][:, j * P:(j + 1) * P],
                        qkro[h][:, q0:q0 + e],
                        start=True, stop=True, skip_group_check=True)
                    if j >= 4 * c:
                        nc.vector.tensor_tensor(
                            sc[:, 0:P], sc[:, 0:P], tri_t[:],
                            op=AluOpType.add)
                    pt = ptl.tile([P, QW], F32R, name="pt", tag="pt")
                    nc.scalar.activation(
                        pt[:, loc0:loc0 + e], sc[:, :e], AF.Exp,
                        bias=dyncol[:, HPC * j + h:HPC * j + h + 1])
                    pts.append((pt, loc0, e))
                    if j >= 1:
                        av_l(j - 1)
                av_l(jmax - 1)
                # l row (partition 0); 1/l columns via tiny transposes
                lr_c = lrc.tile([1, QW], F32, name="lr_c", tag="lr")
                nc.scalar.copy(lr_c[:], lre[0:1, :])
                nc.sync.dma_start(l_d[h:h + 1, c * QW:(c + 1) * QW], lr_c[:])
                arena = dnp.tile([P, QW], F32, name="arena", tag="dn")
                for t in range(4):
                    nc.tensor.transpose(
                        arena[:, t:t + 1],
                        lr_c[0:1, t * P:(t + 1) * P],
                        eye_f[0:1, 0:1])
                nc.vector.reciprocal(linvc[h][:, 4 * c:4 * c + 4],
                                     arena[:, 0:4])
                # attnT copy + scale dance (transpose/scale/transpose back)
                at = atl.tile([P, QW], F32R, name="at", tag="at")
                nc.scalar.copy(at[:], avo[:])
                attnT[(h, c)] = at
                for t in range(4):
                    reg = arena[:, t * P:(t + 1) * P]
                    nc.tensor.transpose(reg.bitcast(F32R),
                                        at[:, t * P:(t + 1) * P], eye_r[:])
                    tm = stg.tile([P, P], F32R, name="tm", tag="tm")
                    nc.vector.tensor_scalar(
                        tm[:], reg,
                        linvc[h][:, 4 * c + t:4 * c + t + 1], None,
                        op0=AluOpType.mult)
                    nc.tensor.transpose(reg.bitcast(F32R), tm[:], eye_r[:])
                    nc.scalar.copy(at[:, t * P:(t + 1) * P], reg)
            # phase E for this chunk: out[q,hid] = sum_h attnT_h(t)^T @ woT_h
            for t in range(4):
                ot = stg.tile([P, HID], F32, name="ot", tag="ot")
                for o in range(4):
                    op = opp.tile([P, QW], F32, name="op", tag="op")
                    for h in range(HPC):
                        nc.tensor.matmul(
                            op[:], attnT[(h, c)][:, t * P:(t + 1) * P],
                            wot[h][:, o * QW:(o + 1) * QW],
                            start=(h == 0), stop=(h == HPC - 1),
                            skip_group_check=True)
                    nc.scalar.copy(ot[:, o * QW:(o + 1) * QW], op[:])
                nc.sync.dma_start(
                    out_d[(4 * c + t) * P:(4 * c + t + 1) * P, :], ot[:])
    ctx.close()


def _host_prep(hidden_states, cos, sin, attention_mask, Wq, Wk, Wv, A, Wdt, Wo):
    eye = np.eye(P, dtype=np.float32)
    perm = np.zeros((P, P), dtype=np.float32)
    for j in range(64):
        perm[j + 64, j] = -1.0
        perm[j, j + 64] = 1.0
    ones128 = np.ones((P, P), dtype=np.float32)
    # causal triangle for the diagonal block, in [k, q] orientation:
    # allowed iff q >= k
    tri = np.where(np.arange(P)[None, :] >= np.arange(P)[:, None],
                   0.0, -BIG).astype(np.float32)

    # verify the attention mask is exactly causal (same for all batches)
    am = np.asarray(attention_mask)
    causal_ref = np.where(np.triu(np.ones((S, S), bool), k=1), MIN, 0.0) \
        .astype(np.float32)
    for b in range(B):
        if not np.array_equal(am[b, 0], causal_ref):
            raise NotImplementedError("non-causal attention mask")

    in_maps = []
    for c in range(NCORES):
        b, g = divmod(c, 4)
        heads = list(range(4 * g, 4 * g + 4))
        wvT = np.ascontiguousarray(Wv[2 * g * D:(2 * g + 2) * D].T)
        wdtvT = np.ascontiguousarray(
            (Wdt[heads].astype(np.float64) @ Wv.astype(np.float64))
            .T.astype(np.float32))
        xT = np.ascontiguousarray(hidden_states[b].T)
        wqT = np.ascontiguousarray(
            (Wq[4 * g * D:(4 * g + 4) * D] * np.float32(SCALING)).T)
        wkT = np.ascontiguousarray(Wk[2 * g * D:(2 * g + 2) * D].T)
        woT = np.ascontiguousarray(Wo[:, 4 * g * D:(4 * g + 4) * D].T)
        acol = A[heads].astype(np.float32).reshape(HPC, 1)
        cosT = np.ascontiguousarray(cos[b].T)
        sinT = np.ascontiguousarray(sin[b].T)
        in_maps.append({
            "xT": xT, "wqT": wqT, "wkT": wkT, "wvT": wvT, "wdtvT": wdtvT,
            "woT": woT, "acol": acol, "cosT": cosT, "sinT": sinT,
            "eye": eye, "perm": perm, "tri": tri, "ones128": ones128,
        })
    return in_maps, None


def _softplus64(x):
    x = x.astype(np.float64)
    return np.log1p(np.exp(-np.abs(x))) + np.maximum(x, 0)


def _repair_rows(out, bad, inputs):
    """Recompute rows flagged bad [B, S] with faithful numpy reference math."""
    if not bad.any():
        return out
    hs = inputs["hidden_states"]; cos = inputs["cos"]; sin = inputs["sin"]
    am = inputs["attention_mask"]; Wq = inputs["Wq"]; Wk = inputs["Wk"]
    Wv = inputs["Wv"]; A = inputs["A"]; Wdt = inputs["Wdt"]; Wo = inputs["Wo"]

    def rope(x, c, s):
        x1, x2 = x[..., :D // 2], x[..., D // 2:]
        return x * c + np.concatenate([-x2, x1], axis=-1) * s

    for b in range(B):
        rows = np.where(bad[b])[0]
        if len(rows) == 0:
            continue
        x = hs[b].astype(np.float32)
        k = (x @ Wk.T).reshape(S, KV, D)
        v = (x @ Wv.T).reshape(S, KV, D)
        k = rope(k, cos[b][:, None, :], sin[b][:, None, :])
        v_flat = v.reshape(S, KV * D)
        dt = v_flat @ Wdt.T
        dyn = np.exp(A[None, :] * _softplus64(dt)).astype(np.float32).T
        kth = np.sort(dyn, axis=-1)[:, NUM_DYN - 1:NUM_DYN]
        dmask = np.where(dyn < kth, MIN, dyn).astype(np.float32)
        for s_i in rows:
            q_row = (x[s_i] @ Wq.T).reshape(H, D)
            q_row = rope(q_row, cos[b][s_i][None, :], sin[b][s_i][None, :])
            attn_row = np.zeros((H, D), dtype=np.float32)
            for h in range(H):
                kvh = h // GROUPS
                sc = ((q_row[h] @ k[:, kvh].T) * np.float32(SCALING)
                      + np.maximum(dmask[h], -BIG)
                      + np.maximum(am[b, 0, s_i], -BIG)).astype(np.float32)
                w = np.exp(sc - sc.max())
                w = (w / w.sum()).astype(np.float32)
                attn_row[h] = w @ v[:, kvh]
            out[b, s_i] = attn_row.reshape(H * D) @ Wo.T
    return out


def kernel(**inputs):
    inputs = {k: np.asarray(v) for k, v in inputs.items()}
    in_maps, blkstate = _host_prep(**inputs)
    nc = _build_program(blkstate)
    res = run_bass_kernel_spmd(nc, in_maps, list(range(NCORES)))
    out = np.zeros((B, S, HID), dtype=np.float32)
    bad = np.zeros((B, S), dtype=bool)
    for c in range(NCORES):
        b = c // 4
        out[b] += res.results[c]["out_q"]
        lc = res.results[c]["l_out"]
        bad[b] |= (lc == 0).any(axis=0)
        bad[b] |= (~np.isfinite(lc)).any(axis=0)
    bad |= ~np.isfinite(out).all(axis=2)
    out = _repair_rows(out, bad, inputs)
    return out


# revision 29
# speedup vs baseline: 1.3655x; 1.0089x over previous
"""DogeDynamicMaskAttention Trainium2 kernel (v2 — transposed attention).

Sharding: 8 cores = 2 batches x 4 head-groups. Core c: batch b=c//4,
head-group g=c%4 -> heads [4g..4g+4), kv heads {2g, 2g+1}.

Device program (SPMD; identical program on all cores, different data):
  - Phase A: dt pre-pass (dt = x @ (Wdt@Wv).T folded on host) streaming x.
  - Phase B: exact per-head kthvalue via 31-step float-bit bisection on DVE
    (drains while phase C runs on PE).
  - Phase C: q/k/v projections with SBUF-resident weights, x streamed once
    per pass as [128,256] tiles; RoPE via f32r permutation-matmul
    rotate-half + DVE combine.
  - Phase D: attention in TRANSPOSED orientation: scT[k,q] = k.q per k-tile,
    causal diag via one constant triangle DVE add, dynamic mask applied as a
    per-partition bias inside the exp activation; attn@v and the softmax
    denominator l (all-ones matmul) accumulate in PSUM over k-tiles.
    Normalization by 1/l via a per-tile transpose/scale/transpose dance.
  - Phase E: output projection out[q,hid] = sum_h attnT_h(t)^T @ WoT_h,
    interleaved per q-chunk with phase D.
  - Degenerate rows (l==0 or non-finite) repaired on host; partials summed
    on host across the 4 head-group cores per batch.
"""
import sys
import numpy as np

sys.path.insert(0, "/root/.axon_site/_ro/trn_rl_repo")

import concourse.bass as bass  # noqa: E402,F401
from concourse import bacc  # noqa: E402
import concourse.tile as tile  # noqa: E402
import concourse.mybir as mybir  # noqa: E402
from concourse.bass_utils import run_bass_kernel_spmd  # noqa: E402
from concourse.alu_op_type import AluOpType  # noqa: E402

F32 = mybir.dt.float32
F32R = mybir.dt.float32r
BF16 = mybir.dt.bfloat16
I32 = mybir.dt.int32
AF = mybir.ActivationFunctionType
AX = mybir.AxisListType.X

B, S, HID = 2, 2048, 2048
H, KV, D = 16, 8, 128
HPC, KVPC = 4, 2
GROUPS = H // KV
NUM_DYN = S // 2
SCALING = D ** -0.5
MIN = float(np.finfo(np.float32).min)
BIG = 1.7e38
P = 128
NT = S // P          # 16 k-tiles
NC = 4               # q chunks of 512
QW = S // NC         # 512
CW = 256             # projection column chunk
NCH = S // CW        # 8 projection chunks
NCORES = 8

_cache = {}


def _build_program(blkstate=None):
    key = "nc"
    if key in _cache:
        return _cache[key]
    nc = bacc.Bacc("TRN2", target_bir_lowering=False, debug=False,
                   num_devices=NCORES)
    dram = {}
    for name, shape, dt_ in [
            ("xT", [HID, S], F32R), ("wqT", [HID, HPC * D], F32R),
            ("wkT", [HID, KVPC * D], F32R), ("wvT", [HID, KVPC * D], F32R),
            ("wdtvT", [HID, HPC], F32R),
            ("woT", [HPC * D, HID], F32R), ("acol", [HPC, 1], F32),
            ("cosT", [D, S], F32), ("sinT", [D, S], F32),
            ("eye", [P, P], F32), ("perm", [P, P], F32), ("tri", [P, P], F32),
            ("ones128", [P, P], F32)]:
        dram[name] = nc.dram_tensor(name, shape, dt_,
                                    kind="ExternalInput").ap()
    out_d = nc.dram_tensor("out_q", [S, HID], F32, kind="ExternalOutput").ap()
    l_d = nc.dram_tensor("l_out", [HPC, S], F32, kind="ExternalOutput").ap()

    with tile.TileContext(nc) as tc:
        _emit(nc, tc, dram, out_d, l_d)
    nc.compile()
    _cache[key] = nc
    return nc


def _emit(nc, tc, dram, out_d, l_d):
    from contextlib import ExitStack
    ctx = ExitStack()
    consts = ctx.enter_context(tc.tile_pool(name="consts", bufs=1))

    def cst(name, shape, as_f32r=False):
        t = consts.tile(shape, F32, name=f"c_{name}")
        nc.sync.dma_start(t[:], dram[name])
        if as_f32r:
            r = consts.tile(shape, F32R, name=f"cr_{name}")
            nc.scalar.copy(r[:], t[:])
            return t, r
        return t

    eye_f, eye_r = cst("eye", [P, P], as_f32r=True)
    _, perm_r = cst("perm", [P, P], as_f32r=True)
    tri_t = cst("tri", [P, P])
    _, ones_r = cst("ones128", [P, P], as_f32r=True)
    acol_t = cst("acol", [HPC, 1])
    # wdtvT packed [128, 16*4]: col cc*4+j = wdtvT[cc*128+p, j]
    wdtv_f = consts.tile([P, NT * HPC], F32R, name="c_wdtvT")
    nc.sync.dma_start(wdtv_f[:].rearrange("p (c j) -> p c j", c=NT),
                      dram["wdtvT"].rearrange("(c p) j -> p c j", p=P))
    kthc = consts.tile([HPC, 1], F32, name="kthc")
    nc.vector.memset(kthc[:], float(NUM_DYN) - 0.5)

    # persistent activation tiles
    act = ctx.enter_context(tc.tile_pool(name="act", bufs=1))
    qkro = [act.tile([P, S], F32R, name=f"qro{h}") for h in range(HPC)]
    kro = [act.tile([P, S], F32R, name=f"kro{i}") for i in range(KVPC)]
    dyncol = act.tile([P, NT * HPC], F32, name="dyncol")  # col 4*j+h
    linvc = [act.tile([P, NT], F32, name=f"linvc{h}") for h in range(HPC)]

    # vT_own lives until the vnat build; kept on ctx (dead afterwards)
    vto = ctx.enter_context(tc.tile_pool(name="vto", bufs=1))
    vT_own = [vto.tile([P, S], F32R, name=f"vTown{i}") for i in range(KVPC)]

    # resident weights; the same space is reused in phase D:
    # wqr -> woT tiles, wkr -> vnat tiles
    wres = ctx.enter_context(tc.tile_pool(name="wres", bufs=1))
    wqr = wres.tile([P, NT * HPC * P], F32R, name="wqr")
    wkr = wres.tile([P, NT * KVPC * P], F32R, name="wkr")
    wvr = wres.tile([P, NT * KVPC * P], F32R, name="wvr")

    dyqa = ExitStack()  # dt/dyn scratch: freed after the vnat build
    dyp = dyqa.enter_context(tc.tile_pool(name="dyqa", bufs=1))
    dt_sb = dyp.tile([HPC, S], F32, name="dt_sb")
    dynm = dyp.tile([HPC, S], F32, name="dynm")

    # ---------------- Phase A: dt pre-pass (streams x) ----------------
    with tc.tile_pool(name="xa", bufs=4) as xa, \
         tc.tile_pool(name="dps", bufs=2, space="PSUM") as dps:
        for sg in range(4):
            dtp = dps.tile([HPC, QW], F32, name="dtp", tag="dtp")
            xh = []
            for part in range(4):
                xt = xa.tile([P, 4 * QW], F32R, name="x32", tag="xa")
                eng = nc.gpsimd if part % 2 == 0 else nc.sync
                eng.dma_start(
                    xt[:].rearrange("p (c f) -> p c f", c=4),
                    dram["xT"][part * 4 * P:(part + 1) * 4 * P,
                               sg * QW:(sg + 1) * QW]
                    .rearrange("(c p) f -> p c f", p=P))
                xh.append(xt)
            for cc in range(NT):
                nc.tensor.matmul(dtp[:], wdtv_f[:, cc * HPC:(cc + 1) * HPC],
                                 xh[cc // 4][:, (cc % 4) * QW:(cc % 4 + 1) * QW],
                                 start=(cc == 0), stop=(cc == NT - 1))
            nc.scalar.copy(dt_sb[:, sg * QW:(sg + 1) * QW], dtp[:])

    # resident weight loads (sync queue; DMA overlaps phase A x stream)
    nc.sync.dma_start(wqr[:].rearrange("p (c f) -> p c f", c=NT),
                      dram["wqT"].rearrange("(c p) f -> p c f", p=P))
    nc.sync.dma_start(wkr[:].rearrange("p (c f) -> p c f", c=NT),
                      dram["wkT"].rearrange("(c p) f -> p c f", p=P))
    nc.sync.dma_start(wvr[:].rearrange("p (c f) -> p c f", c=NT),
                      dram["wvT"].rearrange("(c p) f -> p c f", p=P))

    # ---------------- Phase B: dyn + kth bisection (DVE) -------------
    kth_f = dyp.tile([HPC, 1], I32, name="kth_f")
    dyn_t = dyp.tile([HPC, S], F32, name="dyn_t")
    nc.scalar.activation(dt_sb[:], dt_sb[:], AF.Exp)
    nc.scalar.activation(dt_sb[:], dt_sb[:], AF.Ln, bias=1.0)
    nc.scalar.activation(dyn_t[:], dt_sb[:], AF.Exp, scale=acol_t[:])
    lo = dyp.tile([HPC, 1], I32, name="lo")
    hi = dyp.tile([HPC, 1], I32, name="hi")
    mid = dyp.tile([HPC, 1], I32, name="mid")
    dlt = dyp.tile([HPC, 1], I32, name="dlt")
    cges = dyp.tile([HPC, 1], I32, name="cges")
    cltv = dyp.tile([HPC, 1], I32, name="cltv")
    cnt = dyp.tile([HPC, 1], F32, name="cnt")
    nc.vector.memset(lo[:], 0)
    nc.vector.memset(hi[:], 0x7F800000)
    for _ in range(31):
        nc.vector.tensor_tensor(dlt[:], hi[:], lo[:], op=AluOpType.subtract)
        nc.vector.tensor_scalar(dlt[:], dlt[:], 1, None,
                                op0=AluOpType.arith_shift_right)
        nc.vector.tensor_tensor(mid[:], dlt[:], lo[:], op=AluOpType.add)
        nc.vector.tensor_scalar(dynm[:], dyn_t[:],
                                mid[:, 0:1].bitcast(F32), 0.0,
                                op0=AluOpType.is_lt, op1=AluOpType.add,
                                accum_out=cnt[:])
        nc.vector.tensor_scalar(cges[:], kthc[:], cnt[:, 0:1], None,
                                op0=AluOpType.is_lt)
        nc.vector.tensor_scalar(cltv[:], kthc[:], cnt[:, 0:1], None,
                                op0=AluOpType.is_ge)
        nc.vector.copy_predicated(hi[:], cges[:], mid[:])
        nc.vector.copy_predicated(lo[:], cltv[:], mid[:])
    nc.vector.tensor_copy(kth_f[:], lo[:])
    # dynm = dyn + (dyn < kth) * (-BIG)   (dt_sb reused as scratch)
    nc.vector.tensor_scalar(dt_sb[:], dyn_t[:],
                            kth_f[:, 0:1].bitcast(F32), -BIG,
                            op0=AluOpType.is_lt, op1=AluOpType.mult)
    nc.vector.tensor_tensor(dynm[:], dyn_t[:], dt_sb[:], op=AluOpType.add)

    # ---------------- Phase C: projections + RoPE --------------------
    WQW = HPC * P    # 512 cols per cc block in wqr
    WKW = KVPC * P   # 256 cols per cc block in wkr/wvr
    waves = [[("q", 0), ("q", 1), ("q", 2), ("q", 3)],
             [("k", 0), ("k", 1), ("v", 0), ("v", 1)]]
    with tc.tile_pool(name="xc", bufs=2) as xc, \
         tc.tile_pool(name="cs", bufs=4) as cs, \
         tc.tile_pool(name="pj", bufs=6) as pj, \
         tc.tile_pool(name="pps", bufs=5, space="PSUM") as pps, \
         tc.tile_pool(name="rps", bufs=2, space="PSUM") as rps:
        for ch in range(NCH):
            c0 = ch * CW
            xcf = xc.tile([P, NT * CW], F32R, name="xcf", tag="xc")
            for half in range(2):
                eng = nc.gpsimd if half == 0 else nc.sync
                eng.dma_start(
                    xcf[:, half * 8 * CW:(half + 1) * 8 * CW]
                    .rearrange("p (c f) -> p c f", c=8),
                    dram["xT"][half * 8 * P:(half + 1) * 8 * P, c0:c0 + CW]
                    .rearrange("(c p) f -> p c f", p=P))
            cos_c = cs.tile([D, CW], F32, name="cos_c", tag="cs")
            nc.sync.dma_start(cos_c[:], dram["cosT"][:, c0:c0 + CW])
            sin_c = cs.tile([D, CW], F32, name="sin_c", tag="cs")
            nc.sync.dma_start(sin_c[:], dram["sinT"][:, c0:c0 + CW])
            for wave in waves:
                for kind, oi in wave:
                    if kind == "q":
                        wsrc, ww = wqr, WQW
                    elif kind == "k":
                        wsrc, ww = wkr, WKW
                    else:
                        wsrc, ww = wvr, WKW
                    pp = pps.tile([P, CW], F32, name="pp", tag="pp")
                    for cc in range(NT):
                        nc.tensor.matmul(
                            pp[:],
                            wsrc[:, cc * ww + oi * P:cc * ww + (oi + 1) * P],
                            xcf[:, cc * CW:(cc + 1) * CW],
                            start=(cc == 0), stop=(cc == NT - 1),
                            skip_group_check=True)
                    if kind == "v":
                        nc.scalar.copy(vT_own[oi][:, c0:c0 + CW], pp[:])
                        continue
                    f32t = pj.tile([P, CW], F32R, name="pjr", tag="pj")
                    nc.scalar.copy(f32t[:], pp[:])
                    rh = rps.tile([P, CW], F32, name="rh", tag="rh")
                    nc.tensor.matmul(rh[:], perm_r[:], f32t[:],
                                     start=True, stop=True,
                                     skip_group_check=True)
                    t1 = pj.tile([P, CW], F32, name="t1", tag="pj")
                    nc.vector.tensor_tensor(
                        t1[:], rh[:], sin_c[:], op=AluOpType.mult)
                    t2 = pj.tile([P, CW], F32, name="t2", tag="pj")
                    nc.gpsimd.tensor_tensor(
                        t2[:], f32t[:], cos_c[:], op=AluOpType.mult)
                    dstro = (qkro[oi] if kind == "q" else kro[oi])
                    nc.gpsimd.tensor_tensor(
                        dstro[:, c0:c0 + CW], t1[:], t2[:], op=AluOpType.add)

    # reuse the weight space: woT tiles live in wqr, vnat tiles in wkr
    wot = [wqr[:, h * HID:(h + 1) * HID] for h in range(HPC)]
    for h in range(HPC):
        nc.sync.dma_start(wot[h], dram["woT"][h * P:(h + 1) * P, :])
    vnat = [wkr[:, i * NT * P:(i + 1) * NT * P] for i in range(KVPC)]

    # natural-layout v tiles + dyncol transposes (PE, after phase C)
    with tc.tile_pool(name="vps", bufs=2, space="PSUM") as vps, \
         tc.tile_pool(name="dcp", bufs=1, space="PSUM") as dcp:
        for i in range(KVPC):
            for j in range(NT):
                pt = vps.tile([P, P], F32, name="vt", tag="vt")
                nc.tensor.transpose(pt[:].bitcast(F32R),
                                    vT_own[i][:, j * P:(j + 1) * P],
                                    eye_r[:])
                nc.scalar.copy(vnat[i][:, j * P:(j + 1) * P], pt[:])
        dct = dcp.tile([P, NT * HPC], F32, name="dct")
        for j in range(NT):
            nc.tensor.transpose(dct[:, j * HPC:(j + 1) * HPC],
                                dynm[:, j * P:(j + 1) * P],
                                eye_f[0:HPC, 0:HPC])
        nc.scalar.copy(dyncol[:], dct[:])
    dyqa.close()

    # ---------------- Phase D/E: attention + output projection -------
    attnT = {}
    with tc.tile_pool(name="ptl", bufs=4) as ptl, \
         tc.tile_pool(name="atl", bufs=9) as atl, \
         tc.tile_pool(name="lrc", bufs=3) as lrc, \
         tc.tile_pool(name="stg", bufs=2) as stg, \
         tc.tile_pool(name="scp", bufs=2, space="PSUM") as scp, \
         tc.tile_pool(name="avp", bufs=2, space="PSUM") as avp, \
         tc.tile_pool(name="llp", bufs=1, space="PSUM") as llp, \
         tc.tile_pool(name="dnp", bufs=1, space="PSUM") as dnp, \
         tc.tile_pool(name="opp", bufs=2, space="PSUM") as opp:
        def dance(h, c):
            """Deferred 1/l normalization of attnT[(h, c)] (in place)."""
            at = attnT[(h, c)]
            lr_c = lrows[(h, c)]
            arena = dnp.tile([P, QW], F32, name="arena", tag="dn")
            for t in range(4):
                nc.tensor.transpose(arena[:, t:t + 1],
                                    lr_c[0:1, t * P:(t + 1) * P],
                                    eye_f[0:1, 0:1])
            nc.vector.reciprocal(linvc[h][:, 4 * c:4 * c + 4], arena[:, 0:4])
            for t in range(4):
                reg = arena[:, t * P:(t + 1) * P]
                nc.tensor.transpose(reg.bitcast(F32R),
                                    at[:, t * P:(t + 1) * P], eye_r[:])
                tm = stg.tile([P, P], F32R, name="tm", tag="tm")
                nc.vector.tensor_scalar(
                    tm[:], reg, linvc[h][:, 4 * c + t:4 * c + t + 1], None,
                    op0=AluOpType.mult)
                nc.tensor.transpose(reg.bitcast(F32R), tm[:], eye_r[:])
                nc.scalar.copy(at[:, t * P:(t + 1) * P], reg)

        lrows = {}
        for c in range(NC):
            jmax = 4 * c + 4
            for h in range(HPC):
                kv = h // GROUPS
                avo = avp.tile([P, QW], F32, name="avo", tag="avo")
                lre = llp.tile([P, QW], F32, name="lre", tag="lre")
                pts = []

                def av_l(jj):
                    ptp, pl0, pe_ = pts[jj]
                    nc.tensor.matmul(
                        avo[:, pl0:pl0 + pe_],
                        vnat[kv][:, jj * P:(jj + 1) * P],
                        ptp[:, pl0:pl0 + pe_],
                        start=(jj == 0), stop=(jj == jmax - 1),
                        skip_group_check=True)
                    nc.tensor.matmul(
                        lre[:, pl0:pl0 + pe_], ones_r[:],
                        ptp[:, pl0:pl0 + pe_],
                        start=(jj == 0), stop=(jj == jmax - 1),
                        skip_group_check=True)

                # software-pipelined: av/l for j-1 emitted after kq/exp of j
                for j in range(jmax):
                    q0 = max(c * QW, j * P)
                    e = (c + 1) * QW - q0
                    loc0 = q0 - c * QW
                    sc = scp.tile([P, QW], F32, name="sc", tag="sc")
                    nc.tensor.matmul(
                        sc[:, :e], kro[kv][:, j * P:(j + 1) * P],
                        qkro[h][:, q0:q0 + e],
                        start=True, stop=True, skip_group_check=True)
                    if j >= 4 * c:
                        nc.vector.tensor_tensor(
                            sc[:, 0:P], sc[:, 0:P], tri_t[:],
                            op=AluOpType.add)
                    pt = ptl.tile([P, QW], F32R, name="pt", tag="pt")
                    nc.scalar.activation(
                        pt[:, loc0:loc0 + e], sc[:, :e], AF.Exp,
                        bias=dyncol[:, HPC * j + h:HPC * j + h + 1])
                    pts.append((pt, loc0, e))
                    if j >= 1:
                        av_l(j - 1)
                av_l(jmax - 1)
                # copies out of PSUM (ACT); the 1/l dance is deferred one
                # head so PE stays dense
                lr_c = lrc.tile([1, QW], F32, name="lr_c", tag="lr")
                nc.scalar.copy(lr_c[:], lre[0:1, :])
                nc.sync.dma_start(l_d[h:h + 1, c * QW:(c + 1) * QW], lr_c[:])
                lrows[(h, c)] = lr_c
                at = atl.tile([P, QW], F32R, name="at", tag="at")
                nc.scalar.copy(at[:], avo[:])
                attnT[(h, c)] = at
                if h >= 1:
                    dance(h - 1, c)
            dance(HPC - 1, c)
            # phase E for this chunk: out[q,hid] = sum_h attnT_h(t)^T @ woT_h
            for t in range(4):
                ot = stg.tile([P, HID], F32, name="ot", tag="ot")
                for o in range(4):
                    op = opp.tile([P, QW], F32, name="op", tag="op")
                    for h in range(HPC):
                        nc.tensor.matmul(
                            op[:], attnT[(h, c)][:, t * P:(t + 1) * P],
                            wot[h][:, o * QW:(o + 1) * QW],
                            start=(h == 0), stop=(h == HPC - 1),
                            skip_group_check=True)
                    nc.scalar.copy(ot[:, o * QW:(o + 1) * QW], op[:])
                eng = nc.sync if t % 2 == 0 else nc.gpsimd
                eng.dma_start(
                    out_d[(4 * c + t) * P:(4 * c + t + 1) * P, :], ot[:])
    ctx.close()


def _host_prep(hidden_states, cos, sin, attention_mask, Wq, Wk, Wv, A, Wdt, Wo):
    eye = np.eye(P, dtype=np.float32)
    perm = np.zeros((P, P), dtype=np.float32)
    for j in range(64):
        perm[j + 64, j] = -1.0
        perm[j, j + 64] = 1.0
    ones128 = np.ones((P, P), dtype=np.float32)
    # causal triangle for the diagonal block, in [k, q] orientation:
    # allowed iff q >= k
    tri = np.where(np.arange(P)[None, :] >= np.arange(P)[:, None],
                   0.0, -BIG).astype(np.float32)

    # verify the attention mask is exactly causal (same for all batches)
    am = np.asarray(attention_mask)
    causal_ref = np.where(np.triu(np.ones((S, S), bool), k=1), MIN, 0.0) \
        .astype(np.float32)
    for b in range(B):
        if not np.array_equal(am[b, 0], causal_ref):
            raise NotImplementedError("non-causal attention mask")

    in_maps = []
    for c in range(NCORES):
        b, g = divmod(c, 4)
        heads = list(range(4 * g, 4 * g + 4))
        wvT = np.ascontiguousarray(Wv[2 * g * D:(2 * g + 2) * D].T)
        wdtvT = np.ascontiguousarray(
            (Wdt[heads].astype(np.float64) @ Wv.astype(np.float64))
            .T.astype(np.float32))
        xT = np.ascontiguousarray(hidden_states[b].T)
        wqT = np.ascontiguousarray(
            (Wq[4 * g * D:(4 * g + 4) * D] * np.float32(SCALING)).T)
        wkT = np.ascontiguousarray(Wk[2 * g * D:(2 * g + 2) * D].T)
        woT = np.ascontiguousarray(Wo[:, 4 * g * D:(4 * g + 4) * D].T)
        acol = A[heads].astype(np.float32).reshape(HPC, 1)
        cosT = np.ascontiguousarray(cos[b].T)
        sinT = np.ascontiguousarray(sin[b].T)
        in_maps.append({
            "xT": xT, "wqT": wqT, "wkT": wkT, "wvT": wvT, "wdtvT": wdtvT,
            "woT": woT, "acol": acol, "cosT": cosT, "sinT": sinT,
            "eye": eye, "perm": perm, "tri": tri, "ones128": ones128,
        })
    return in_maps, None


def _softplus64(x):
    x = x.astype(np.float64)
    return np.log1p(np.exp(-np.abs(x))) + np.maximum(x, 0)


def _repair_rows(out, bad, inputs):
    """Recompute rows flagged bad [B, S] with faithful numpy reference math."""
    if not bad.any():
        return out
    hs = inputs["hidden_states"]; cos = inputs["cos"]; sin = inputs["sin"]
    am = inputs["attention_mask"]; Wq = inputs["Wq"]; Wk = inputs["Wk"]
    Wv = inputs["Wv"]; A = inputs["A"]; Wdt = inputs["Wdt"]; Wo = inputs["Wo"]

    def rope(x, c, s):
        x1, x2 = x[..., :D // 2], x[..., D // 2:]
        return x * c + np.concatenate([-x2, x1], axis=-1) * s

    for b in range(B):
        rows = np.where(bad[b])[0]
        if len(rows) == 0:
            continue
        x = hs[b].astype(np.float32)
        k = (x @ Wk.T).reshape(S, KV, D)
        v = (x @ Wv.T).reshape(S, KV, D)
        k = rope(k, cos[b][:, None, :], sin[b][:, None, :])
        v_flat = v.reshape(S, KV * D)
        dt = v_flat @ Wdt.T
        dyn = np.exp(A[None, :] * _softplus64(dt)).astype(np.float32).T
        kth = np.sort(dyn, axis=-1)[:, NUM_DYN - 1:NUM_DYN]
        dmask = np.where(dyn < kth, MIN, dyn).astype(np.float32)
        for s_i in rows:
            q_row = (x[s_i] @ Wq.T).reshape(H, D)
            q_row = rope(q_row, cos[b][s_i][None, :], sin[b][s_i][None, :])
            attn_row = np.zeros((H, D), dtype=np.float32)
            for h in range(H):
                kvh = h // GROUPS
                sc = ((q_row[h] @ k[:, kvh].T) * np.float32(SCALING)
                      + np.maximum(dmask[h], -BIG)
                      + np.maximum(am[b, 0, s_i], -BIG)).astype(np.float32)
                w = np.exp(sc - sc.max())
                w = (w / w.sum()).astype(np.float32)
                attn_row[h] = w @ v[:, kvh]
            out[b, s_i] = attn_row.reshape(H * D) @ Wo.T
    return out


def kernel(**inputs):
    inputs = {k: np.asarray(v) for k, v in inputs.items()}
    in_maps, blkstate = _host_prep(**inputs)
    nc = _build_program(blkstate)
    res = run_bass_kernel_spmd(nc, in_maps, list(range(NCORES)))
    out = np.zeros((B, S, HID), dtype=np.float32)
    bad = np.zeros((B, S), dtype=bool)
    for c in range(NCORES):
        b = c // 4
        out[b] += res.results[c]["out_q"]
        lc = res.results[c]["l_out"]
        bad[b] |= (lc == 0).any(axis=0)
        bad[b] |= (~np.isfinite(lc)).any(axis=0)
    bad |= ~np.isfinite(out).all(axis=2)
    out = _repair_rows(out, bad, inputs)
    return out


# revision 37
# speedup vs baseline: 1.6754x; 1.2269x over previous
"""DogeDynamicMaskAttention Trainium2 kernel (v2 — transposed attention).

Sharding: 8 cores = 2 batches x 4 head-groups. Core c: batch b=c//4,
head-group g=c%4 -> heads [4g..4g+4), kv heads {2g, 2g+1}.

Device program (SPMD; identical program on all cores, different data):
  - Phase A: dt pre-pass (dt = x @ (Wdt@Wv).T folded on host) streaming x.
  - Phase B: exact per-head kthvalue via 31-step float-bit bisection on DVE
    (drains while phase C runs on PE).
  - Phase C: q/k/v projections with SBUF-resident weights, x streamed once
    per pass as [128,256] tiles; RoPE via f32r permutation-matmul
    rotate-half + DVE combine.
  - Phase D: attention in TRANSPOSED orientation: scT[k,q] = k.q per k-tile,
    causal diag via one constant triangle DVE add, dynamic mask applied as a
    per-partition bias inside the exp activation; attn@v and the softmax
    denominator l (all-ones matmul) accumulate in PSUM over k-tiles.
    Normalization by 1/l via a per-tile transpose/scale/transpose dance.
  - Phase E: output projection out[q,hid] = sum_h attnT_h(t)^T @ WoT_h,
    interleaved per q-chunk with phase D.
  - Degenerate rows (l==0 or non-finite) repaired on host; partials summed
    on host across the 4 head-group cores per batch.
"""
import sys
import numpy as np

sys.path.insert(0, "/root/.axon_site/_ro/trn_rl_repo")

import concourse.bass as bass  # noqa: E402,F401
from concourse import bacc  # noqa: E402
import concourse.tile as tile  # noqa: E402
import concourse.mybir as mybir  # noqa: E402
from concourse.bass_utils import run_bass_kernel_spmd  # noqa: E402
from concourse.alu_op_type import AluOpType  # noqa: E402

F32 = mybir.dt.float32
F32R = mybir.dt.float32r
BF16 = mybir.dt.bfloat16
I32 = mybir.dt.int32
AF = mybir.ActivationFunctionType
AX = mybir.AxisListType.X

B, S, HID = 2, 2048, 2048
H, KV, D = 16, 8, 128
HPC, KVPC = 4, 2
GROUPS = H // KV
NUM_DYN = S // 2
SCALING = D ** -0.5
MIN = float(np.finfo(np.float32).min)
BIG = 1.7e38
P = 128
NT = S // P          # 16 k-tiles
NC = 4               # q chunks of 512
QW = S // NC         # 512
CW = 256             # projection column chunk
NCH = S // CW        # 8 projection chunks
NCORES = 8

_cache = {}


def _build_program(blkstate=None):
    key = "nc"
    if key in _cache:
        return _cache[key]
    nc = bacc.Bacc("TRN2", target_bir_lowering=False, debug=False,
                   num_devices=NCORES)
    dram = {}
    for name, shape, dt_ in [
            ("xT", [HID, S], F32R), ("wqT", [HID, HPC * D], F32R),
            ("wkT", [HID, KVPC * D], F32R), ("wvT", [HID, KVPC * D], F32R),
            ("wdtvT", [HID, HPC], F32R),
            ("woT", [HPC * D, HID], F32R), ("acol", [HPC, 1], F32),
            ("cosT", [D, S], F32), ("sinT", [D, S], F32),
            ("eye", [P, P], F32), ("perm", [P, P], F32), ("tri", [P, P], F32),
            ("ones128", [P, P], F32)]:
        dram[name] = nc.dram_tensor(name, shape, dt_,
                                    kind="ExternalInput").ap()
    out_d = nc.dram_tensor("out_q", [S, HID], F32, kind="ExternalOutput").ap()
    l_d = nc.dram_tensor("l_out", [HPC, S], F32, kind="ExternalOutput").ap()

    with tile.TileContext(nc) as tc:
        _emit(nc, tc, dram, out_d, l_d)
    nc.compile()
    _cache[key] = nc
    return nc


def _emit(nc, tc, dram, out_d, l_d):
    from contextlib import ExitStack
    ctx = ExitStack()
    consts = ctx.enter_context(tc.tile_pool(name="consts", bufs=1))

    def cst(name, shape, as_f32r=False):
        t = consts.tile(shape, F32, name=f"c_{name}")
        nc.sync.dma_start(t[:], dram[name])
        if as_f32r:
            r = consts.tile(shape, F32R, name=f"cr_{name}")
            nc.scalar.copy(r[:], t[:])
            return t, r
        return t

    eye_f, eye_r = cst("eye", [P, P], as_f32r=True)
    _, perm_r = cst("perm", [P, P], as_f32r=True)
    tri_t = cst("tri", [P, P])
    _, ones_r = cst("ones128", [P, P], as_f32r=True)
    acol_t = cst("acol", [HPC, 1])
    # wdtvT packed [128, 16*4]: col cc*4+j = wdtvT[cc*128+p, j]
    wdtv_f = consts.tile([P, NT * HPC], F32R, name="c_wdtvT")
    nc.sync.dma_start(wdtv_f[:].rearrange("p (c j) -> p c j", c=NT),
                      dram["wdtvT"].rearrange("(c p) j -> p c j", p=P))
    kthc = consts.tile([HPC, 1], F32, name="kthc")
    nc.vector.memset(kthc[:], float(NUM_DYN) - 0.5)

    # persistent activation tiles
    act = ctx.enter_context(tc.tile_pool(name="act", bufs=1))
    qkro = [act.tile([P, S], F32R, name=f"qro{h}") for h in range(HPC)]
    kro = [act.tile([P, S], F32R, name=f"kro{i}") for i in range(KVPC)]
    dyncol = act.tile([P, NT * HPC], F32, name="dyncol")  # col 4*j+h
    linvc = [act.tile([P, NT], F32, name=f"linvc{h}") for h in range(HPC)]

    # vT_own lives until the vnat build; kept on ctx (dead afterwards)
    vto = ctx.enter_context(tc.tile_pool(name="vto", bufs=1))
    vT_own = [vto.tile([P, S], F32R, name=f"vTown{i}") for i in range(KVPC)]

    # resident weights; the same space is reused in phase D:
    # wqr -> woT tiles, wkr -> vnat tiles
    wres = ctx.enter_context(tc.tile_pool(name="wres", bufs=1))
    wqr = wres.tile([P, NT * HPC * P], F32R, name="wqr")
    wkr = wres.tile([P, NT * KVPC * P], F32R, name="wkr")
    wvr = wres.tile([P, NT * KVPC * P], F32R, name="wvr")

    dyqa = ExitStack()  # dt/dyn scratch: freed after the vnat build
    dyp = dyqa.enter_context(tc.tile_pool(name="dyqa", bufs=1))
    dt_sb = dyp.tile([HPC, S], F32, name="dt_sb")
    dynm = dyp.tile([HPC, S], F32, name="dynm")

    # ---------------- Phase A: dt pre-pass (streams x) ----------------
    with tc.tile_pool(name="xa", bufs=4) as xa, \
         tc.tile_pool(name="dps", bufs=2, space="PSUM") as dps:
        for sg in range(4):
            dtp = dps.tile([HPC, QW], F32, name="dtp", tag="dtp")
            xh = []
            for part in range(4):
                xt = xa.tile([P, 4 * QW], F32R, name="x32", tag="xa")
                eng = nc.gpsimd if part % 2 == 0 else nc.sync
                eng.dma_start(
                    xt[:].rearrange("p (c f) -> p c f", c=4),
                    dram["xT"][part * 4 * P:(part + 1) * 4 * P,
                               sg * QW:(sg + 1) * QW]
                    .rearrange("(c p) f -> p c f", p=P))
                xh.append(xt)
            for cc in range(NT):
                nc.tensor.matmul(dtp[:], wdtv_f[:, cc * HPC:(cc + 1) * HPC],
                                 xh[cc // 4][:, (cc % 4) * QW:(cc % 4 + 1) * QW],
                                 start=(cc == 0), stop=(cc == NT - 1))
            nc.scalar.copy(dt_sb[:, sg * QW:(sg + 1) * QW], dtp[:])

    # resident weight loads (sync queue; DMA overlaps phase A x stream)
    nc.sync.dma_start(wqr[:].rearrange("p (c f) -> p c f", c=NT),
                      dram["wqT"].rearrange("(c p) f -> p c f", p=P))
    nc.sync.dma_start(wkr[:].rearrange("p (c f) -> p c f", c=NT),
                      dram["wkT"].rearrange("(c p) f -> p c f", p=P))
    nc.sync.dma_start(wvr[:].rearrange("p (c f) -> p c f", c=NT),
                      dram["wvT"].rearrange("(c p) f -> p c f", p=P))

    # ---------------- Phase B: dyn + kth bisection (DVE) -------------
    # Greedy MSB-first prefix bisection on float bits: lo accumulates the
    # largest value with count(dyn < lo) <= 1023, which is exactly the
    # 1024-th smallest element (kthvalue).
    dyn_t = dyp.tile([HPC, S], F32, name="dyn_t")
    nc.scalar.activation(dt_sb[:], dt_sb[:], AF.Exp)
    nc.scalar.activation(dt_sb[:], dt_sb[:], AF.Ln, bias=1.0)
    nc.scalar.activation(dyn_t[:], dt_sb[:], AF.Exp, scale=acol_t[:])
    lo = dyp.tile([HPC, 1], I32, name="lo")
    tst = dyp.tile([HPC, 1], I32, name="tst")
    bit = dyp.tile([HPC, 1], I32, name="bit")
    keep = dyp.tile([HPC, 1], I32, name="keep")
    cnt = dyp.tile([HPC, 1], F32, name="cnt")
    nc.vector.memset(lo[:], 0)
    for k in range(30, -1, -1):
        nc.vector.memset(bit[:], 1 << k)
        nc.vector.tensor_tensor(tst[:], lo[:], bit[:], op=AluOpType.add)
        nc.vector.tensor_scalar(dt_sb[:], dyn_t[:],
                                tst[:, 0:1].bitcast(F32), 0.0,
                                op0=AluOpType.is_lt, op1=AluOpType.add,
                                accum_out=cnt[:])
        nc.vector.tensor_scalar(keep[:], kthc[:], cnt[:, 0:1], None,
                                op0=AluOpType.is_ge)
        nc.vector.copy_predicated(lo[:], keep[:], tst[:])
    # dynm = dyn + (dyn < kth) * (-BIG)   (dt_sb reused as scratch)
    nc.vector.tensor_scalar(dt_sb[:], dyn_t[:],
                            lo[:, 0:1].bitcast(F32), -BIG,
                            op0=AluOpType.is_lt, op1=AluOpType.mult)
    nc.vector.tensor_tensor(dynm[:], dyn_t[:], dt_sb[:], op=AluOpType.add)

    # ---------------- Phase C: projections + RoPE --------------------
    WQW = HPC * P    # 512 cols per cc block in wqr
    WKW = KVPC * P   # 256 cols per cc block in wkr/wvr
    waves = [[("q", 0), ("q", 1), ("q", 2), ("q", 3)],
             [("k", 0), ("k", 1), ("v", 0), ("v", 1)]]
    rope_alt = [0]
    with tc.tile_pool(name="xc", bufs=2) as xc, \
         tc.tile_pool(name="cs", bufs=4) as cs, \
         tc.tile_pool(name="pj", bufs=6) as pj, \
         tc.tile_pool(name="pps", bufs=5, space="PSUM") as pps, \
         tc.tile_pool(name="rps", bufs=1, space="PSUM") as rps, \
         tc.tile_pool(name="vps", bufs=2, space="PSUM") as vps:

        def fetch(ch):
            c0 = ch * CW
            xcf = xc.tile([P, NT * CW], F32R, name="xcf", tag="xc")
            for half in range(2):
                eng = nc.gpsimd if half == 0 else nc.sync
                eng.dma_start(
                    xcf[:, half * 8 * CW:(half + 1) * 8 * CW]
                    .rearrange("p (c f) -> p c f", c=8),
                    dram["xT"][half * 8 * P:(half + 1) * 8 * P, c0:c0 + CW]
                    .rearrange("(c p) f -> p c f", p=P))
            cos_c = cs.tile([D, CW], F32, name="cos_c", tag="cs")
            nc.sync.dma_start(cos_c[:], dram["cosT"][:, c0:c0 + CW])
            sin_c = cs.tile([D, CW], F32, name="sin_c", tag="cs")
            nc.sync.dma_start(sin_c[:], dram["sinT"][:, c0:c0 + CW])
            return xcf, cos_c, sin_c

        nxt = fetch(0)
        for ch in range(NCH):
            c0 = ch * CW
            xcf, cos_c, sin_c = nxt
            if ch + 1 < NCH:
                nxt = fetch(ch + 1)
            for wave in waves:
                for kind, oi in wave:
                    if kind == "q":
                        wsrc, ww = wqr, WQW
                    elif kind == "k":
                        wsrc, ww = wkr, WKW
                    else:
                        wsrc, ww = wvr, WKW
                    pp = pps.tile([P, CW], F32, name="pp", tag="pp")
                    for cc in range(NT):
                        nc.tensor.matmul(
                            pp[:],
                            wsrc[:, cc * ww + oi * P:cc * ww + (oi + 1) * P],
                            xcf[:, cc * CW:(cc + 1) * CW],
                            start=(cc == 0), stop=(cc == NT - 1),
                            skip_group_check=True)
                    if kind == "v":
                        nc.scalar.copy(vT_own[oi][:, c0:c0 + CW], pp[:])
                        # in-place transpose to natural [k, d] layout for
                        # the two k-tiles this chunk covers
                        for jj in range(2):
                            j = 2 * ch + jj
                            pt = vps.tile([P, P], F32, name="vt", tag="vt")
                            nc.tensor.transpose(
                                pt[:].bitcast(F32R),
                                vT_own[oi][:, j * P:(j + 1) * P], eye_r[:])
                            nc.scalar.copy(
                                vT_own[oi][:, j * P:(j + 1) * P], pt[:])
                        continue
                    f32t = pj.tile([P, CW], F32R, name="pjr", tag="pj")
                    nc.scalar.copy(f32t[:], pp[:])
                    rh = rps.tile([P, CW], F32, name="rh", tag="rh")
                    nc.tensor.matmul(rh[:], perm_r[:], f32t[:],
                                     start=True, stop=True,
                                     skip_group_check=True)
                    t1 = pj.tile([P, CW], F32, name="t1", tag="pj")
                    nc.vector.tensor_tensor(
                        t1[:], rh[:], sin_c[:], op=AluOpType.mult)
                    t2 = pj.tile([P, CW], F32, name="t2", tag="pj")
                    nc.gpsimd.tensor_tensor(
                        t2[:], f32t[:], cos_c[:], op=AluOpType.mult)
                    dstro = (qkro[oi] if kind == "q" else kro[oi])
                    eng = nc.vector if rope_alt[0] % 2 == 0 else nc.gpsimd
                    rope_alt[0] += 1
                    eng.tensor_tensor(
                        dstro[:, c0:c0 + CW], t1[:], t2[:], op=AluOpType.add)

    # after the in-place transposes, vT_own holds the natural [k, d] tiles
    vnat = vT_own

    # reuse the weight space: woT tiles live in wqr
    wot = [wqr[:, h * HID:(h + 1) * HID] for h in range(HPC)]
    for h in range(HPC):
        nc.sync.dma_start(wot[h], dram["woT"][h * P:(h + 1) * P, :])

    # dyncol transposes (PE, after bisection)
    with tc.tile_pool(name="dcp", bufs=1, space="PSUM") as dcp:
        dct = dcp.tile([P, NT * HPC], F32, name="dct")
        for j in range(NT):
            nc.tensor.transpose(dct[:, j * HPC:(j + 1) * HPC],
                                dynm[:, j * P:(j + 1) * P],
                                eye_f[0:HPC, 0:HPC])
        nc.scalar.copy(dyncol[:], dct[:])
    dyqa.close()

    # ---------------- Phase D/E: attention + output projection -------
    attnT = {}
    with tc.tile_pool(name="ptl", bufs=4) as ptl, \
         tc.tile_pool(name="atl", bufs=9) as atl, \
         tc.tile_pool(name="lrc", bufs=3) as lrc, \
         tc.tile_pool(name="stg", bufs=2) as stg, \
         tc.tile_pool(name="scp", bufs=2, space="PSUM") as scp, \
         tc.tile_pool(name="avp", bufs=2, space="PSUM") as avp, \
         tc.tile_pool(name="llp", bufs=1, space="PSUM") as llp, \
         tc.tile_pool(name="dnp", bufs=1, space="PSUM") as dnp, \
         tc.tile_pool(name="opp", bufs=2, space="PSUM") as opp:
        def dance(h, c):
            """Deferred 1/l normalization of attnT[(h, c)] (in place)."""
            at = attnT[(h, c)]
            lr_c = lrows[(h, c)]
            arena = dnp.tile([P, QW], F32, name="arena", tag="dn")
            for t in range(4):
                nc.tensor.transpose(arena[:, t:t + 1],
                                    lr_c[0:1, t * P:(t + 1) * P],
                                    eye_f[0:1, 0:1])
            nc.vector.reciprocal(linvc[h][:, 4 * c:4 * c + 4], arena[:, 0:4])
            for t in range(4):
                reg = arena[:, t * P:(t + 1) * P]
                nc.tensor.transpose(reg.bitcast(F32R),
                                    at[:, t * P:(t + 1) * P], eye_r[:])
                tm = stg.tile([P, P], F32R, name="tm", tag="tm")
                nc.vector.tensor_scalar(
                    tm[:], reg, linvc[h][:, 4 * c + t:4 * c + t + 1], None,
                    op0=AluOpType.mult)
                nc.tensor.transpose(reg.bitcast(F32R), tm[:], eye_r[:])
                nc.scalar.copy(at[:, t * P:(t + 1) * P], reg)

        lrows = {}
        for c in range(NC):
            jmax = 4 * c + 4
            for h in range(HPC):
                kv = h // GROUPS
                avo = avp.tile([P, QW], F32, name="avo", tag="avo")
                lre = llp.tile([P, QW], F32, name="lre", tag="lre")
                pts = []

                def av_l(jj):
                    ptp, pl0, pe_ = pts[jj]
                    nc.tensor.matmul(
                        avo[:, pl0:pl0 + pe_],
                        vnat[kv][:, jj * P:(jj + 1) * P],
                        ptp[:, pl0:pl0 + pe_],
                        start=(jj == 0), stop=(jj == jmax - 1),
                        skip_group_check=True)
                    nc.tensor.matmul(
                        lre[:, pl0:pl0 + pe_], ones_r[:],
                        ptp[:, pl0:pl0 + pe_],
                        start=(jj == 0), stop=(jj == jmax - 1),
                        skip_group_check=True)

                # software-pipelined: av/l for j-1 emitted after kq/exp of j
                for j in range(jmax):
                    q0 = max(c * QW, j * P)
                    e = (c + 1) * QW - q0
                    loc0 = q0 - c * QW
                    sc = scp.tile([P, QW], F32, name="sc", tag="sc")
                    nc.tensor.matmul(
                        sc[:, :e], kro[kv][:, j * P:(j + 1) * P],
                        qkro[h][:, q0:q0 + e],
                        start=True, stop=True, skip_group_check=True)
                    if j >= 4 * c:
                        nc.vector.tensor_tensor(
                            sc[:, 0:P], sc[:, 0:P], tri_t[:],
                            op=AluOpType.add)
                    pt = ptl.tile([P, QW], F32R, name="pt", tag="pt")
                    nc.scalar.activation(
                        pt[:, loc0:loc0 + e], sc[:, :e], AF.Exp,
                        bias=dyncol[:, HPC * j + h:HPC * j + h + 1])
                    pts.append((pt, loc0, e))
                    if j >= 1:
                        av_l(j - 1)
                av_l(jmax - 1)
                # copies out of PSUM (ACT); the 1/l dance is deferred one
                # head so PE stays dense
                lr_c = lrc.tile([1, QW], F32, name="lr_c", tag="lr")
                nc.scalar.copy(lr_c[:], lre[0:1, :])
                nc.sync.dma_start(l_d[h:h + 1, c * QW:(c + 1) * QW], lr_c[:])
                lrows[(h, c)] = lr_c
                at = atl.tile([P, QW], F32R, name="at", tag="at")
                nc.scalar.copy(at[:], avo[:])
                attnT[(h, c)] = at
                if h >= 1:
                    dance(h - 1, c)
            dance(HPC - 1, c)
            # phase E for this chunk: out[q,hid] = sum_h attnT_h(t)^T @ woT_h
            for t in range(4):
                ot = stg.tile([P, HID], F32, name="ot", tag="ot")
                for o in range(4):
                    op = opp.tile([P, QW], F32, name="op", tag="op")
                    for h in range(HPC):
                        nc.tensor.matmul(
                            op[:], attnT[(h, c)][:, t * P:(t + 1) * P],
                            wot[h][:, o * QW:(o + 1) * QW],
                            start=(h == 0), stop=(h == HPC - 1),
                            skip_group_check=True)
                    nc.scalar.copy(ot[:, o * QW:(o + 1) * QW], op[:])
                eng = nc.sync if t % 2 == 0 else nc.gpsimd
                eng.dma_start(
                    out_d[(4 * c + t) * P:(4 * c + t + 1) * P, :], ot[:])
    ctx.close()


def _host_prep(hidden_states, cos, sin, attention_mask, Wq, Wk, Wv, A, Wdt, Wo):
    eye = np.eye(P, dtype=np.float32)
    perm = np.zeros((P, P), dtype=np.float32)
    for j in range(64):
        perm[j + 64, j] = -1.0
        perm[j, j + 64] = 1.0
    ones128 = np.ones((P, P), dtype=np.float32)
    # causal triangle for the diagonal block, in [k, q] orientation:
    # allowed iff q >= k
    tri = np.where(np.arange(P)[None, :] >= np.arange(P)[:, None],
                   0.0, -BIG).astype(np.float32)

    # verify the attention mask is exactly causal (same for all batches)
    am = np.asarray(attention_mask)
    causal_ref = np.where(np.triu(np.ones((S, S), bool), k=1), MIN, 0.0) \
        .astype(np.float32)
    for b in range(B):
        if not np.array_equal(am[b, 0], causal_ref):
            raise NotImplementedError("non-causal attention mask")

    in_maps = []
    for c in range(NCORES):
        b, g = divmod(c, 4)
        heads = list(range(4 * g, 4 * g + 4))
        wvT = np.ascontiguousarray(Wv[2 * g * D:(2 * g + 2) * D].T)
        wdtvT = np.ascontiguousarray(
            (Wdt[heads].astype(np.float64) @ Wv.astype(np.float64))
            .T.astype(np.float32))
        xT = np.ascontiguousarray(hidden_states[b].T)
        wqT = np.ascontiguousarray(
            (Wq[4 * g * D:(4 * g + 4) * D] * np.float32(SCALING)).T)
        wkT = np.ascontiguousarray(Wk[2 * g * D:(2 * g + 2) * D].T)
        woT = np.ascontiguousarray(Wo[:, 4 * g * D:(4 * g + 4) * D].T)
        acol = A[heads].astype(np.float32).reshape(HPC, 1)
        cosT = np.ascontiguousarray(cos[b].T)
        sinT = np.ascontiguousarray(sin[b].T)
        in_maps.append({
            "xT": xT, "wqT": wqT, "wkT": wkT, "wvT": wvT, "wdtvT": wdtvT,
            "woT": woT, "acol": acol, "cosT": cosT, "sinT": sinT,
            "eye": eye, "perm": perm, "tri": tri, "ones128": ones128,
        })
    return in_maps, None


def _softplus64(x):
    x = x.astype(np.float64)
    return np.log1p(np.exp(-np.abs(x))) + np.maximum(x, 0)


def _repair_rows(out, bad, inputs):
    """Recompute rows flagged bad [B, S] with faithful numpy reference math."""
    if not bad.any():
        return out
    hs = inputs["hidden_states"]; cos = inputs["cos"]; sin = inputs["sin"]
    am = inputs["attention_mask"]; Wq = inputs["Wq"]; Wk = inputs["Wk"]
    Wv = inputs["Wv"]; A = inputs["A"]; Wdt = inputs["Wdt"]; Wo = inputs["Wo"]

    def rope(x, c, s):
        x1, x2 = x[..., :D // 2], x[..., D // 2:]
        return x * c + np.concatenate([-x2, x1], axis=-1) * s

    for b in range(B):
        rows = np.where(bad[b])[0]
        if len(rows) == 0:
            continue
        x = hs[b].astype(np.float32)
        k = (x @ Wk.T).reshape(S, KV, D)
        v = (x @ Wv.T).reshape(S, KV, D)
        k = rope(k, cos[b][:, None, :], sin[b][:, None, :])
        v_flat = v.reshape(S, KV * D)
        dt = v_flat @ Wdt.T
        dyn = np.exp(A[None, :] * _softplus64(dt)).astype(np.float32).T
        kth = np.sort(dyn, axis=-1)[:, NUM_DYN - 1:NUM_DYN]
        dmask = np.where(dyn < kth, MIN, dyn).astype(np.float32)
        for s_i in rows:
            q_row = (x[s_i] @ Wq.T).reshape(H, D)
            q_row = rope(q_row, cos[b][s_i][None, :], sin[b][s_i][None, :])
            attn_row = np.zeros((H, D), dtype=np.float32)
            for h in range(H):
                kvh = h // GROUPS
                sc = ((q_row[h] @ k[:, kvh].T) * np.float32(SCALING)
                      + np.maximum(dmask[h], -BIG)
                      + np.maximum(am[b, 0, s_i], -BIG)).astype(np.float32)
                w = np.exp(sc - sc.max())
                w = (w / w.sum()).astype(np.float32)
                attn_row[h] = w @ v[:, kvh]
            out[b, s_i] = attn_row.reshape(H * D) @ Wo.T
    return out


def kernel(**inputs):
    inputs = {k: np.asarray(v) for k, v in inputs.items()}
    in_maps, blkstate = _host_prep(**inputs)
    nc = _build_program(blkstate)
    res = run_bass_kernel_spmd(nc, in_maps, list(range(NCORES)))
    out = np.zeros((B, S, HID), dtype=np.float32)
    bad = np.zeros((B, S), dtype=bool)
    for c in range(NCORES):
        b = c // 4
        out[b] += res.results[c]["out_q"]
        lc = res.results[c]["l_out"]
        bad[b] |= (lc == 0).any(axis=0)
        bad[b] |= (~np.isfinite(lc)).any(axis=0)
    bad |= ~np.isfinite(out).all(axis=2)
    out = _repair_rows(out, bad, inputs)
    return out
